# revision 32
# baseline (speedup 1.0000x reference)
"""Causal multi-head attention block on 8 Trainium2 NeuronCores.

Sharding: 8 cores = 4 batches (data parallel) x 2 head-groups (tensor
parallel over heads). Core c handles batch c//2 and global heads
(c%2)*8 .. (c%2)*8+8. Each core computes a partial output projection
(split-K over its 512 head-output channels); the host sums the two
partials per batch and adds b_proj.

Per-core kernel (bf16 operands, fp32 PSUM accumulation):
  inputs:  x = x^T [1024, 2048] bf16 (host pre-transposes the batch),
           wqkv [1152, 1536] bf16 (rows 0..1023 = w_attn cols for this
           core's q|k|v heads, row 1024 = b_attn slice, rest zero),
           wproj [512, 1024] bf16
  output:  out [2048, 1024] fp32 = partial projection

Design notes (vs the fp32r baseline this evolved from):
  - x arrives pre-transposed; x^T strips are contiguous DMA loads.
  - All matmul operands are bf16: 1 cycle/row at any N (exact causal
    trimming of diagonal tiles), and FWL fast weight loads.
  - S^T tiles [j=128, head-pair, i=512] fp32 psum; one Exp per tile.
  - PV uses M=128 stationary [v_h (64 cols) | ones (64 cols)]: rows
    64..127 of the PV psum replicate the softmax denominator, so the
    reciprocal runs as one custom-DVE reciprocal_approx_fast (ACT
    Ln/Exp would thrash activation table sets; plain DVE reciprocal
    is ~6.4ns/elem/lane).
  - b_attn for the q|k strips folds into the psum evacuation as a
    per-partition tensor_scalar_add; the v strip keeps the x_aug
    ones-row augmentation.
  - qkT strips are emitted q0,k0,q1,k1,... so attention for head-pair
    0 overlaps the rest of the qkv projection.
"""

import threading
from contextlib import ExitStack

import numpy as np
import ml_dtypes

import concourse.bass as bass
import concourse.mybir as mybir
import concourse.tile as tile
from concourse import bacc
from concourse.bass_utils import run_bass_kernel_spmd

F32 = mybir.dt.float32
BF16 = mybir.dt.bfloat16
NP_BF16 = ml_dtypes.bfloat16

B, T, C = 4, 2048, 1024
H, DH = 16, 64
N_CORES = 8
HL = 8                  # local heads per core
NQK = 2 * HL * DH       # 1024 qkT rows (q 512 | k 512)
NV = HL * DH            # 512 v cols
CS = C // 128           # 8 real c-strips
CS_AUG = CS + 1         # + bias strip
TT = T // 128           # 16 token tiles
TB = T // 512           # 4 token blocks
SCALE = 1.0 / 8.0       # 1/sqrt(DH)
ACT_EXP = mybir.ActivationFunctionType.Exp


def build_attention_kernel(ctx: ExitStack, tc: tile.TileContext,
                           x: bass.AP, wqkv: bass.AP, wproj: bass.AP,
                           bqk: bass.AP, out: bass.AP):
    nc = tc.nc

    const_pool = ctx.enter_context(tc.tile_pool(name="const", bufs=1))
    # x_aug^T bias strip: row 0 ones, rows 1..127 zero.
    ones_strip = const_pool.tile([128, 512], BF16, tag="ones")
    nc.gpsimd.memset(ones_strip[:], 0.0)
    nc.gpsimd.memset(ones_strip[0:1, :], 1.0)
    # causal diag mask: 1 where i >= j (keep), 0 where i < j
    mask01 = const_pool.tile([128, 128], BF16, tag="mask01")
    nc.gpsimd.memset(mask01[:], 1.0)
    nc.gpsimd.affine_select(
        out=mask01[:], in_=mask01[:],
        compare_op=mybir.AluOpType.is_ge, fill=0.0, base=0,
        pattern=[[1, 128]], channel_multiplier=-1)

    # persistent SBUF
    qkt_pool = ctx.enter_context(tc.tile_pool(name="qkt", bufs=1))
    qkt = [qkt_pool.tile([128, T], BF16, tag=f"qkt{s}", name=f"qkt{s}")
           for s in range(NQK // 128)]
    vau_pool = ctx.enter_context(tc.tile_pool(name="vau", bufs=1))
    # [j, h, 0:64] = ones (denominator replicator; base-0 so the
    # custom-DVE reciprocal reads PSUM partitions 0..63 -- a shifted
    # base corrupts InstCustomDveAnt); [j, h, 64:128] = v_h
    vau = [vau_pool.tile([128, HL, 2 * DH], BF16, tag=f"v{tt}",
                         name=f"vau{tt}")
           for tt in range(TT)]
    for tt in range(TT):
        nc.gpsimd.memset(vau[tt][:, :, 0:DH], 1.0)
    yt_pool = ctx.enter_context(tc.tile_pool(name="yt", bufs=1))
    yt = [yt_pool.tile([128, T], BF16, tag=f"yt{s}", name=f"yt{s}")
          for s in range(NV // 128)]

    # b_attn per-partition bias columns for the q|k strips
    bias_qk = const_pool.tile([128, 8], F32, tag="biasqk")
    nc.sync.dma_start(bias_qk[:], bqk.rearrange("(s p) -> p s", p=128))

    # ---- phases 2+3 interleaved: qkv projection, attention, proj ----
    # PE executes its instruction stream in order, so overlap between
    # the PE-dense qkv chains and the ACT-paced attention must be woven
    # into the emission order: strips q0,k0 -> first attention block ->
    # more strips -> ... Later, vau chains and the previous i-block's
    # projection chunks fill PE while exp paces attention.
    wp_pool = ctx.enter_context(tc.tile_pool(name="wp", bufs=1))
    wp = wp_pool.tile([128, NV // 128, C], BF16, tag="wp")
    nc.sync.dma_start(wp[:], wproj.rearrange("(s p) n -> p s n", p=128))
    wv_pool = ctx.enter_context(tc.tile_pool(name="wv", bufs=1))
    wv = wv_pool.tile([128, CS_AUG, NV], BF16, tag="wv")
    for s in range(CS_AUG):  # per-strip so the first chains start early
        nc.sync.dma_start(wv[:, s, :], wqkv[s * 128:(s + 1) * 128, NQK:])

    pt_sb_pool = ctx.enter_context(tc.tile_pool(name="ptile", bufs=3))
    n_sb_pool = ctx.enter_context(tc.tile_pool(name="ntile", bufs=2))
    osb_pool = ctx.enter_context(tc.tile_pool(name="osb", bufs=1))
    ps_s_pool = ctx.enter_context(
        tc.tile_pool(name="ps_s", bufs=2, space="PSUM"))
    ps_y_pool = ctx.enter_context(
        tc.tile_pool(name="ps_y", bufs=1, space="PSUM"))

    # x^T strips: freed once the last qkv chains are done
    xt_ctx = ExitStack()
    xt_pool = xt_ctx.enter_context(tc.tile_pool(name="xt", bufs=1))
    xt = [xt_pool.tile([128, T], BF16, tag=f"xt{s}", name=f"xt{s}")
          for s in range(CS)]
    for s in range(CS):
        nc.sync.dma_start(xt[s][:], x[s * 128:(s + 1) * 128, :])

    wnn_ctx = ExitStack()
    wnn_pool = wnn_ctx.enter_context(tc.tile_pool(name="wnn", bufs=2))
    pqk_pool = wnn_ctx.enter_context(
        tc.tile_pool(name="pqk", bufs=1, space="PSUM"))
    pv_pool = wnn_ctx.enter_context(
        tc.tile_pool(name="pv", bufs=1, space="PSUM"))

    def qk_strip(nn):
        wn = wnn_pool.tile([128, CS_AUG, 128], BF16, tag="wnn")
        nc.sync.dma_start(
            wn[:],
            wqkv[:, nn * 128:(nn + 1) * 128]
            .rearrange("(s p) n -> p s n", p=128))
        for tb in range(TB):
            ps = pqk_pool.tile([128, 512], F32, tag="pqk")
            for s in range(CS):
                nc.tensor.matmul(ps[:], wn[:, s, :],
                                 xt[s][:, tb * 512:(tb + 1) * 512],
                                 start=(s == 0), stop=(s == CS - 1))
            # evacuate with the b_attn bias folded in (per-partition)
            nc.vector.tensor_scalar_add(
                qkt[nn][:, tb * 512:(tb + 1) * 512], ps[:],
                bias_qk[:, nn:nn + 1])

    def vau_tile(tt):
        ps = pv_pool.tile([128, NV], F32, tag="pv")
        for s in range(CS_AUG):
            lhsT = (ones_strip[:, 0:128] if s == CS
                    else xt[s][:, tt * 128:(tt + 1) * 128])
            nc.tensor.matmul(ps[:], lhsT, wv[:, s, :],
                             start=(s == 0), stop=(s == CS_AUG - 1))
        nc.vector.tensor_copy(
            vau[tt][:, :, DH:],
            ps[:].rearrange("p (h d) -> p h d", d=DH))

    def attn_block(ib, hp):
        isl = slice(ib * 512, (ib + 1) * 512)
        jmax = 4 * ib + 3
        qs = qkt[hp]              # q strip: heads (2hp, 2hp+1)
        ks = qkt[4 + hp]          # k strip
        ps_y = [ps_y_pool.tile([128, 512], F32, tag=f"psy{u}",
                               name=f"psy{u}_{hp}_{ib}")
                for u in range(2)]
        for jj in range(jmax + 1):
            off = max(0, 128 * (jj - 4 * ib))
            ps_s = ps_s_pool.tile([128, 2, 512], F32, tag="pss")
            for u in range(2):     # head-pair halves: base 0 / 64
                plo = 64 * u
                nc.tensor.matmul(
                    ps_s[:, u, off:],
                    ks[plo:plo + DH, jj * 128:(jj + 1) * 128],
                    qs[plo:plo + DH, ib * 512 + off:(ib + 1) * 512],
                    start=True, stop=True)
            p = pt_sb_pool.tile([128, 2, 512], BF16, tag="pt")
            nc.scalar.activation(p[:, :, off:], ps_s[:, :, off:],
                                 ACT_EXP, scale=SCALE)
            if jj >= 4 * ib:       # diagonal tile: zero i < j
                nc.vector.tensor_mul(
                    p[:, :, off:off + 128],
                    p[:, :, off:off + 128],
                    mask01[:, None, :].broadcast_to([128, 2, 128]))
            for u in range(2):
                nc.tensor.matmul(ps_y[u][:, off:],
                                 vau[jj][:, 2 * hp + u, :],
                                 p[:, u, off:],
                                 start=(jj == 0), stop=(jj == jmax))
        for u in range(2):
            plo = 64 * u
            rbb = n_sb_pool.tile([64, 512], F32, tag=f"rbb{u}")
            nc.vector.reciprocal_approx_fast(
                out=rbb[:], in_=ps_y[u][0:64, :])
            nc.vector.tensor_mul(yt[hp][plo:plo + DH, isl],
                                 ps_y[u][64:128, :], rbb[:])

    def proj_chunk(tt, nb):
        # psum borrows the psy slots: 8 banks = 4 ps_s + 2 psy + 1 pqk
        # + 1 pv
        ps = ps_y_pool.tile([128, 512], F32, tag=f"psy{(tt + nb) % 2}",
                            name=f"po{tt}_{nb}")
        for s in range(NV // 128):
            nc.tensor.matmul(
                ps[:],
                yt[s][:, tt * 128:(tt + 1) * 128],
                wp[:, s, nb * 512:(nb + 1) * 512],
                start=(s == 0), stop=(s == NV // 128 - 1))
        osl = slice(nb * 512, (nb + 1) * 512)
        o_sb = osb[tt % 2]
        if (tt + nb) % 2 == 0:
            nc.scalar.copy(o_sb[:, osl], ps[:])
        else:
            nc.vector.tensor_copy(o_sb[:, osl], ps[:])
        if nb == C // 512 - 1:
            nc.sync.dma_start(out[tt * 128:(tt + 1) * 128, :], o_sb[:])

    # -- weave --
    qk_strip(0)
    qk_strip(4)
    for tt in range(4):
        vau_tile(tt)
    attn_block(0, 0)
    qk_strip(1)
    qk_strip(5)
    attn_block(0, 1)
    qk_strip(2)
    qk_strip(6)
    attn_block(0, 2)
    qk_strip(3)
    qk_strip(7)
    attn_block(0, 3)
    for tt in range(4, TT):
        vau_tile(tt)
    wnn_ctx.close()
    xt_ctx.close()  # release x^T strips

    osb = [osb_pool.tile([128, C], F32, tag=f"osb{i}", name=f"osb{i}")
           for i in range(2)]
    for ib in range(1, TB):
        for hp in range(HL // 2):
            attn_block(ib, hp)
            # previous i-block's projection as PE filler
            base = 4 * (ib - 1)
            for tt, nb in ((base + hp, 0), (base + hp, 1)):
                proj_chunk(tt, nb)
    for tt in range(4 * (TB - 1), T // 128):
        for nb in range(C // 512):
            proj_chunk(tt, nb)


_BUILD_LOCK = threading.Lock()
_CACHED = {}


def build_nc(repeat=1):
    with _BUILD_LOCK:
        if repeat in _CACHED:
            return _CACHED[repeat]
        nc = bacc.Bacc("TRN2", debug=False)
        x = nc.dram_tensor("x", [C, T], BF16, kind="ExternalInput").ap()
        wqkv = nc.dram_tensor("wqkv", [CS_AUG * 128, 3 * NV], BF16,
                              kind="ExternalInput").ap()
        wproj = nc.dram_tensor("wproj", [NV, C], BF16,
                               kind="ExternalInput").ap()
        bqk = nc.dram_tensor("bqk", [NQK], F32, kind="ExternalInput").ap()
        out = nc.dram_tensor("out", [T, C], F32, kind="ExternalOutput").ap()
        with tile.TileContext(nc, pool_alloc_mode="queue") as tc:
            for _ in range(repeat):
                with ExitStack() as ctx:
                    build_attention_kernel(ctx, tc, x, wqkv, wproj, bqk, out)
        nc.compile()
        _CACHED[repeat] = nc
        return nc


def shard_inputs(x, w_attn, b_attn, w_proj, b_proj):
    """Build the per-core input maps (numpy, bf16)."""
    x = np.asarray(x, dtype=np.float32)
    w_attn = np.asarray(w_attn, dtype=np.float32)
    b_attn = np.asarray(b_attn, dtype=np.float32)
    w_proj = np.asarray(w_proj, dtype=np.float32)
    in_maps = []
    for c in range(N_CORES):
        b, hh = divmod(c, 2)
        cols = np.r_[hh * 512:(hh + 1) * 512,
                     C + hh * 512:C + (hh + 1) * 512,
                     2 * C + hh * 512:2 * C + (hh + 1) * 512]
        w_aug = np.zeros((CS_AUG * 128, 3 * NV), np.float32)
        w_aug[:C] = w_attn[:, cols]
        w_aug[C] = b_attn[cols]
        in_maps.append({
            "x": np.ascontiguousarray(x[b].T).astype(NP_BF16),
            "wqkv": w_aug.astype(NP_BF16),
            "wproj": np.ascontiguousarray(
                w_proj[hh * 512:(hh + 1) * 512]).astype(NP_BF16),
            "bqk": np.ascontiguousarray(b_attn[cols[:NQK]]),
        })
    return in_maps


def kernel(x, w_attn, b_attn, w_proj, b_proj, _profile=False, _tmpdir=None):
    nc = build_nc()
    in_maps = shard_inputs(x, w_attn, b_attn, w_proj, b_proj)
    res = run_bass_kernel_spmd(nc, in_maps, list(range(N_CORES)),
                               trace=_profile, tmpdir=_tmpdir)
    b_proj = np.asarray(b_proj, dtype=np.float32)
    out = np.empty((B, T, C), np.float32)
    for b in range(B):
        out[b] = res.results[2 * b]["out"] + res.results[2 * b + 1]["out"] \
            + b_proj[None, :]
    if _profile:
        return out, res
    return out


# revision 36
# speedup vs baseline: 1.0120x; 1.0120x over previous
"""Causal multi-head attention block on 8 Trainium2 NeuronCores.

Sharding: 8 cores = 4 batches (data parallel) x 2 head-groups (tensor
parallel over heads). Core c handles batch c//2 and global heads
(c%2)*8 .. (c%2)*8+8. Each core computes a partial output projection
(split-K over its 512 head-output channels); the host sums the two
partials per batch and adds b_proj.

Per-core kernel (bf16 operands, fp32 PSUM accumulation):
  inputs:  x = x^T [1024, 2048] bf16 (host pre-transposes the batch),
           wqkv [1152, 1536] bf16 (rows 0..1023 = w_attn cols for this
           core's q|k|v heads, row 1024 = b_attn slice, rest zero),
           wproj [512, 1024] bf16
  output:  out [2048, 1024] fp32 = partial projection

Design notes (vs the fp32r baseline this evolved from):
  - x arrives pre-transposed; x^T strips are contiguous DMA loads.
  - All matmul operands are bf16: 1 cycle/row at any N (exact causal
    trimming of diagonal tiles), and FWL fast weight loads.
  - S^T tiles [j=128, head-pair, i=512] fp32 psum; one Exp per tile.
  - PV uses M=128 stationary [v_h (64 cols) | ones (64 cols)]: rows
    64..127 of the PV psum replicate the softmax denominator, so the
    reciprocal runs as one custom-DVE reciprocal_approx_fast (ACT
    Ln/Exp would thrash activation table sets; plain DVE reciprocal
    is ~6.4ns/elem/lane).
  - b_attn for the q|k strips folds into the psum evacuation as a
    per-partition tensor_scalar_add; the v strip keeps the x_aug
    ones-row augmentation.
  - qkT strips are emitted q0,k0,q1,k1,... so attention for head-pair
    0 overlaps the rest of the qkv projection.
"""

import threading
from contextlib import ExitStack

import numpy as np
import ml_dtypes

import concourse.bass as bass
import concourse.mybir as mybir
import concourse.tile as tile
from concourse import bacc
from concourse.bass_utils import run_bass_kernel_spmd

F32 = mybir.dt.float32
BF16 = mybir.dt.bfloat16
NP_BF16 = ml_dtypes.bfloat16

B, T, C = 4, 2048, 1024
H, DH = 16, 64
N_CORES = 8
HL = 8                  # local heads per core
NQK = 2 * HL * DH       # 1024 qkT rows (q 512 | k 512)
NV = HL * DH            # 512 v cols
CS = C // 128           # 8 real c-strips
CS_AUG = CS + 1         # + bias strip
TT = T // 128           # 16 token tiles
TB = T // 512           # 4 token blocks
SCALE = 1.0 / 8.0       # 1/sqrt(DH)
ACT_EXP = mybir.ActivationFunctionType.Exp


def build_attention_kernel(ctx: ExitStack, tc: tile.TileContext,
                           x: bass.AP, wqkv: bass.AP, wproj: bass.AP,
                           bqk: bass.AP, out: bass.AP):
    nc = tc.nc

    const_pool = ctx.enter_context(tc.tile_pool(name="const", bufs=1))
    # x_aug^T bias strip: row 0 ones, rows 1..127 zero.
    ones_strip = const_pool.tile([128, 512], BF16, tag="ones")
    nc.gpsimd.memset(ones_strip[:], 0.0)
    nc.gpsimd.memset(ones_strip[0:1, :], 1.0)
    # causal diag mask: 1 where i >= j (keep), 0 where i < j
    mask01 = const_pool.tile([128, 128], BF16, tag="mask01")
    nc.gpsimd.memset(mask01[:], 1.0)
    nc.gpsimd.affine_select(
        out=mask01[:], in_=mask01[:],
        compare_op=mybir.AluOpType.is_ge, fill=0.0, base=0,
        pattern=[[1, 128]], channel_multiplier=-1)

    # persistent SBUF
    qkt_pool = ctx.enter_context(tc.tile_pool(name="qkt", bufs=1))
    qkt = [qkt_pool.tile([128, T], BF16, tag=f"qkt{s}", name=f"qkt{s}")
           for s in range(NQK // 128)]
    vau_pool = ctx.enter_context(tc.tile_pool(name="vau", bufs=1))
    # [j, h, 0:64] = ones (denominator replicator; base-0 so the
    # custom-DVE reciprocal reads PSUM partitions 0..63 -- a shifted
    # base corrupts InstCustomDveAnt); [j, h, 64:128] = v_h
    vau = [vau_pool.tile([128, HL, 2 * DH], BF16, tag=f"v{tt}",
                         name=f"vau{tt}")
           for tt in range(TT)]
    for tt in range(TT):
        nc.gpsimd.memset(vau[tt][:, :, 0:DH], 1.0)
    yt_pool = ctx.enter_context(tc.tile_pool(name="yt", bufs=1))
    yt = [yt_pool.tile([128, T], BF16, tag=f"yt{s}", name=f"yt{s}")
          for s in range(NV // 128)]

    # b_attn per-partition bias columns for the q|k strips
    bias_qk = const_pool.tile([128, 8], F32, tag="biasqk")
    nc.sync.dma_start(bias_qk[:], bqk.rearrange("(s p) -> p s", p=128))

    # ---- phases 2+3 interleaved: qkv projection, attention, proj ----
    # PE executes its instruction stream in order, so overlap between
    # the PE-dense qkv chains and the ACT-paced attention must be woven
    # into the emission order: strips q0,k0 -> first attention block ->
    # more strips -> ... Later, vau chains and the previous i-block's
    # projection chunks fill PE while exp paces attention.
    wp_pool = ctx.enter_context(tc.tile_pool(name="wp", bufs=1))
    wp = wp_pool.tile([128, NV // 128, C], BF16, tag="wp")
    nc.sync.dma_start(wp[:], wproj.rearrange("(s p) n -> p s n", p=128))
    wv_pool = ctx.enter_context(tc.tile_pool(name="wv", bufs=1))
    wv = wv_pool.tile([128, CS_AUG, NV], BF16, tag="wv")
    for s in range(CS_AUG):  # per-strip so the first chains start early
        nc.sync.dma_start(wv[:, s, :], wqkv[s * 128:(s + 1) * 128, NQK:])

    pt_sb_pool = ctx.enter_context(tc.tile_pool(name="ptile", bufs=3))
    n_sb_pool = ctx.enter_context(tc.tile_pool(name="ntile", bufs=2))
    osb_pool = ctx.enter_context(tc.tile_pool(name="osb", bufs=1))

    # phase-A pools (weave: qkv strips + vau + attention ib=0):
    # PSUM = 4 (ps_sA) + 2 (ps_yA) + 1 (pqk) + 1 (pv) = 8 banks
    wnn_ctx = ExitStack()
    ps_s_pool = wnn_ctx.enter_context(
        tc.tile_pool(name="ps_sA", bufs=2, space="PSUM"))
    ps_y_pool = wnn_ctx.enter_context(
        tc.tile_pool(name="ps_yA", bufs=1, space="PSUM"))
    xt_pool = wnn_ctx.enter_context(tc.tile_pool(name="xt", bufs=1))
    xt = [xt_pool.tile([128, T], BF16, tag=f"xt{s}", name=f"xt{s}")
          for s in range(CS)]
    for s in range(CS):
        nc.sync.dma_start(xt[s][:], x[s * 128:(s + 1) * 128, :])
    wnn_pool = wnn_ctx.enter_context(tc.tile_pool(name="wnn", bufs=2))
    pqk_pool = wnn_ctx.enter_context(
        tc.tile_pool(name="pqk", bufs=1, space="PSUM"))
    pv_pool = wnn_ctx.enter_context(
        tc.tile_pool(name="pv", bufs=1, space="PSUM"))

    def qk_strip(nn):
        wn = wnn_pool.tile([128, CS_AUG, 128], BF16, tag="wnn")
        nc.sync.dma_start(
            wn[:],
            wqkv[:, nn * 128:(nn + 1) * 128]
            .rearrange("(s p) n -> p s n", p=128))
        for tb in range(TB):
            ps = pqk_pool.tile([128, 512], F32, tag="pqk")
            for s in range(CS):
                nc.tensor.matmul(ps[:], wn[:, s, :],
                                 xt[s][:, tb * 512:(tb + 1) * 512],
                                 start=(s == 0), stop=(s == CS - 1))
            # evacuate with the b_attn bias folded in (per-partition)
            nc.vector.tensor_scalar_add(
                qkt[nn][:, tb * 512:(tb + 1) * 512], ps[:],
                bias_qk[:, nn:nn + 1])

    def vau_tile(tt):
        ps = pv_pool.tile([128, NV], F32, tag="pv")
        for s in range(CS_AUG):
            lhsT = (ones_strip[:, 0:128] if s == CS
                    else xt[s][:, tt * 128:(tt + 1) * 128])
            nc.tensor.matmul(ps[:], lhsT, wv[:, s, :],
                             start=(s == 0), stop=(s == CS_AUG - 1))
        nc.vector.tensor_copy(
            vau[tt][:, :, DH:],
            ps[:].rearrange("p (h d) -> p h d", d=DH))

    def attn_block(ib, hp, psy_par=""):
        isl = slice(ib * 512, (ib + 1) * 512)
        jmax = 4 * ib + 3
        qs = qkt[hp]              # q strip: heads (2hp, 2hp+1)
        ks = qkt[4 + hp]          # k strip
        ps_y = [ps_y_pool.tile([128, 512], F32, tag=f"psy{u}{psy_par}",
                               name=f"psy{u}_{hp}_{ib}")
                for u in range(2)]

        def s_exp(jj):
            # S^T pair + exp + diagonal mask for one j-tile
            off = max(0, 128 * (jj - 4 * ib))
            ps_s = ps_s_pool.tile([128, 2, 512], F32, tag="pss")
            for u in range(2):     # head-pair halves: base 0 / 64
                plo = 64 * u
                nc.tensor.matmul(
                    ps_s[:, u, off:],
                    ks[plo:plo + DH, jj * 128:(jj + 1) * 128],
                    qs[plo:plo + DH, ib * 512 + off:(ib + 1) * 512],
                    start=True, stop=True)
            p = pt_sb_pool.tile([128, 2, 512], BF16, tag="pt")
            nc.scalar.activation(p[:, :, off:], ps_s[:, :, off:],
                                 ACT_EXP, scale=SCALE)
            if jj >= 4 * ib:       # diagonal tile: zero i < j
                nc.vector.tensor_mul(
                    p[:, :, off:off + 128],
                    p[:, :, off:off + 128],
                    mask01[:, None, :].broadcast_to([128, 2, 128]))
            return p

        def pv(jj, p):
            off = max(0, 128 * (jj - 4 * ib))
            for u in range(2):
                nc.tensor.matmul(ps_y[u][:, off:],
                                 vau[jj][:, 2 * hp + u, :],
                                 p[:, u, off:],
                                 start=(jj == 0), stop=(jj == jmax))

        # software pipeline: issue S(jj+1) before PV(jj) so the PE has
        # independent work queued while ACT finishes exp(jj) -- avoids
        # the isolated-matmul drain penalty on the first PV.
        p_prev = s_exp(0)
        for jj in range(1, jmax + 1):
            p_cur = s_exp(jj)
            pv(jj - 1, p_prev)
            p_prev = p_cur
        pv(jmax, p_prev)
        for u in range(2):
            plo = 64 * u
            rbb = n_sb_pool.tile([64, 512], F32, tag=f"rbb{u}")
            nc.vector.reciprocal_approx_fast(
                out=rbb[:], in_=ps_y[u][0:64, :])
            nc.vector.tensor_mul(yt[hp][plo:plo + DH, isl],
                                 ps_y[u][64:128, :], rbb[:])

    def proj_chunk(tt, nb):
        # psum borrows the psy slots: 8 banks = 4 ps_s + 2 psy + 1 pqk
        # + 1 pv
        ps = ps_y_pool.tile([128, 512], F32, tag=f"psy{nb}{tt % 2}",
                            name=f"po{tt}_{nb}")
        for s in range(NV // 128):
            nc.tensor.matmul(
                ps[:],
                yt[s][:, tt * 128:(tt + 1) * 128],
                wp[:, s, nb * 512:(nb + 1) * 512],
                start=(s == 0), stop=(s == NV // 128 - 1))
        osl = slice(nb * 512, (nb + 1) * 512)
        o_sb = osb[tt % 2]
        if (tt + nb) % 2 == 0:
            nc.scalar.copy(o_sb[:, osl], ps[:])
        else:
            nc.vector.tensor_copy(o_sb[:, osl], ps[:])
        if nb == C // 512 - 1:
            nc.sync.dma_start(out[tt * 128:(tt + 1) * 128, :], o_sb[:])

    # -- weave --
    qk_strip(0)
    qk_strip(4)
    for tt in range(4):
        vau_tile(tt)
    attn_block(0, 0)
    qk_strip(1)
    qk_strip(5)
    attn_block(0, 1)
    qk_strip(2)
    qk_strip(6)
    attn_block(0, 2)
    qk_strip(3)
    qk_strip(7)
    attn_block(0, 3)
    for tt in range(4, TT):
        vau_tile(tt)
    wnn_ctx.close()  # release x^T strips + phase-A psum pools

    # phase-B pools: 4 (ps_sB) + 4 (ps_yB, u x parity, shared with
    # proj) = 8 banks
    ps_s_pool = ctx.enter_context(
        tc.tile_pool(name="ps_sB", bufs=2, space="PSUM"))
    ps_y_pool = ctx.enter_context(
        tc.tile_pool(name="ps_yB", bufs=1, space="PSUM"))

    osb = [osb_pool.tile([128, C], F32, tag=f"osb{i}", name=f"osb{i}")
           for i in range(2)]
    for ib in range(1, TB):
        for hp in range(HL // 2):
            attn_block(ib, hp, psy_par=str(hp % 2))
            # previous i-block's projection as PE filler
            base = 4 * (ib - 1)
            for tt, nb in ((base + hp, 0), (base + hp, 1)):
                proj_chunk(tt, nb)
    for tt in range(4 * (TB - 1), T // 128):
        for nb in range(C // 512):
            proj_chunk(tt, nb)


_BUILD_LOCK = threading.Lock()
_CACHED = {}


def build_nc(repeat=1):
    with _BUILD_LOCK:
        if repeat in _CACHED:
            return _CACHED[repeat]
        nc = bacc.Bacc("TRN2", debug=False)
        x = nc.dram_tensor("x", [C, T], BF16, kind="ExternalInput").ap()
        wqkv = nc.dram_tensor("wqkv", [CS_AUG * 128, 3 * NV], BF16,
                              kind="ExternalInput").ap()
        wproj = nc.dram_tensor("wproj", [NV, C], BF16,
                               kind="ExternalInput").ap()
        bqk = nc.dram_tensor("bqk", [NQK], F32, kind="ExternalInput").ap()
        out = nc.dram_tensor("out", [T, C], F32, kind="ExternalOutput").ap()
        with tile.TileContext(nc, pool_alloc_mode="queue") as tc:
            for _ in range(repeat):
                with ExitStack() as ctx:
                    build_attention_kernel(ctx, tc, x, wqkv, wproj, bqk, out)
        nc.compile()
        _CACHED[repeat] = nc
        return nc


def shard_inputs(x, w_attn, b_attn, w_proj, b_proj):
    """Build the per-core input maps (numpy, bf16)."""
    x = np.asarray(x, dtype=np.float32)
    w_attn = np.asarray(w_attn, dtype=np.float32)
    b_attn = np.asarray(b_attn, dtype=np.float32)
    w_proj = np.asarray(w_proj, dtype=np.float32)
    in_maps = []
    for c in range(N_CORES):
        b, hh = divmod(c, 2)
        cols = np.r_[hh * 512:(hh + 1) * 512,
                     C + hh * 512:C + (hh + 1) * 512,
                     2 * C + hh * 512:2 * C + (hh + 1) * 512]
        w_aug = np.zeros((CS_AUG * 128, 3 * NV), np.float32)
        w_aug[:C] = w_attn[:, cols]
        w_aug[C] = b_attn[cols]
        in_maps.append({
            "x": np.ascontiguousarray(x[b].T).astype(NP_BF16),
            "wqkv": w_aug.astype(NP_BF16),
            "wproj": np.ascontiguousarray(
                w_proj[hh * 512:(hh + 1) * 512]).astype(NP_BF16),
            "bqk": np.ascontiguousarray(b_attn[cols[:NQK]]),
        })
    return in_maps


def kernel(x, w_attn, b_attn, w_proj, b_proj, _profile=False, _tmpdir=None):
    nc = build_nc()
    in_maps = shard_inputs(x, w_attn, b_attn, w_proj, b_proj)
    res = run_bass_kernel_spmd(nc, in_maps, list(range(N_CORES)),
                               trace=_profile, tmpdir=_tmpdir)
    b_proj = np.asarray(b_proj, dtype=np.float32)
    out = np.empty((B, T, C), np.float32)
    for b in range(B):
        out[b] = res.results[2 * b]["out"] + res.results[2 * b + 1]["out"] \
            + b_proj[None, :]
    if _profile:
        return out, res
    return out


# revision 38
# speedup vs baseline: 1.0499x; 1.0374x over previous
"""Causal multi-head attention block on 8 Trainium2 NeuronCores.

Sharding: 8 cores = 4 batches (data parallel) x 2 head-groups (tensor
parallel over heads). Core c handles batch c//2 and global heads
(c%2)*8 .. (c%2)*8+8. Each core computes a partial output projection
(split-K over its 512 head-output channels); the host sums the two
partials per batch and adds b_proj.

Per-core kernel (bf16 operands, fp32 PSUM accumulation):
  inputs:  x = x^T [1024, 2048] bf16 (host pre-transposes the batch),
           wqkv [1152, 1536] bf16 (rows 0..1023 = w_attn cols for this
           core's q|k|v heads, row 1024 = b_attn slice, rest zero),
           wproj [512, 1024] bf16
  output:  out [2048, 1024] fp32 = partial projection

Design notes (vs the fp32r baseline this evolved from):
  - x arrives pre-transposed; x^T strips are contiguous DMA loads.
  - All matmul operands are bf16: 1 cycle/row at any N (exact causal
    trimming of diagonal tiles), and FWL fast weight loads.
  - S^T tiles [j=128, head-pair, i=512] fp32 psum; one Exp per tile.
  - PV uses M=128 stationary [v_h (64 cols) | ones (64 cols)]: rows
    64..127 of the PV psum replicate the softmax denominator, so the
    reciprocal runs as one custom-DVE reciprocal_approx_fast (ACT
    Ln/Exp would thrash activation table sets; plain DVE reciprocal
    is ~6.4ns/elem/lane).
  - b_attn for the q|k strips folds into the psum evacuation as a
    per-partition tensor_scalar_add; the v strip keeps the x_aug
    ones-row augmentation.
  - qkT strips are emitted q0,k0,q1,k1,... so attention for head-pair
    0 overlaps the rest of the qkv projection.
"""

import threading
from contextlib import ExitStack

import numpy as np
import ml_dtypes

import concourse.bass as bass
import concourse.mybir as mybir
import concourse.tile as tile
from concourse import bacc
from concourse.bass_utils import run_bass_kernel_spmd

F32 = mybir.dt.float32
BF16 = mybir.dt.bfloat16
NP_BF16 = ml_dtypes.bfloat16

B, T, C = 4, 2048, 1024
H, DH = 16, 64
N_CORES = 8
HL = 8                  # local heads per core
NQK = 2 * HL * DH       # 1024 qkT rows (q 512 | k 512)
NV = HL * DH            # 512 v cols
CS = C // 128           # 8 real c-strips
CS_AUG = CS + 1         # + bias strip
TT = T // 128           # 16 token tiles
TB = T // 512           # 4 token blocks
SCALE = 1.0 / 8.0       # 1/sqrt(DH)
ACT_EXP = mybir.ActivationFunctionType.Exp


def build_attention_kernel(ctx: ExitStack, tc: tile.TileContext,
                           x: bass.AP, wqkv: bass.AP, wproj: bass.AP,
                           bqk: bass.AP, out: bass.AP):
    nc = tc.nc

    const_pool = ctx.enter_context(tc.tile_pool(name="const", bufs=1))
    # x_aug^T bias strip: row 0 ones, rows 1..127 zero.
    ones_strip = const_pool.tile([128, 512], BF16, tag="ones")
    nc.gpsimd.memset(ones_strip[:], 0.0)
    nc.gpsimd.memset(ones_strip[0:1, :], 1.0)
    # causal diag mask: 1 where i >= j (keep), 0 where i < j
    mask01 = const_pool.tile([128, 128], BF16, tag="mask01")
    nc.gpsimd.memset(mask01[:], 1.0)
    nc.gpsimd.affine_select(
        out=mask01[:], in_=mask01[:],
        compare_op=mybir.AluOpType.is_ge, fill=0.0, base=0,
        pattern=[[1, 128]], channel_multiplier=-1)

    # persistent SBUF
    qkt_pool = ctx.enter_context(tc.tile_pool(name="qkt", bufs=1))
    qkt = [qkt_pool.tile([128, T], BF16, tag=f"qkt{s}", name=f"qkt{s}")
           for s in range(NQK // 128)]
    vau_pool = ctx.enter_context(tc.tile_pool(name="vau", bufs=1))
    # [j, h, 0:64] = ones (denominator replicator; base-0 so the
    # custom-DVE reciprocal reads PSUM partitions 0..63 -- a shifted
    # base corrupts InstCustomDveAnt); [j, h, 64:128] = v_h
    vau = [vau_pool.tile([128, HL, 2 * DH], BF16, tag=f"v{tt}",
                         name=f"vau{tt}")
           for tt in range(TT)]
    for tt in range(TT):
        nc.gpsimd.memset(vau[tt][:, :, 0:DH], 1.0)
    yt_pool = ctx.enter_context(tc.tile_pool(name="yt", bufs=1))
    yt = [yt_pool.tile([128, T], BF16, tag=f"yt{s}", name=f"yt{s}")
          for s in range(NV // 128)]

    # ---- phases 1-2 share the x^T strips; freed before attention ----
    xt_ctx = ExitStack()
    xt_pool = xt_ctx.enter_context(tc.tile_pool(name="xt", bufs=1))
    xt = [xt_pool.tile([128, T], BF16, tag=f"xt{s}", name=f"xt{s}")
          for s in range(CS)]

    # ---- phase 1: x^T strips (x is pre-transposed host-side) ----
    for s in range(CS):
        nc.sync.dma_start(xt[s][:], x[s * 128:(s + 1) * 128, :])
    # b_attn per-partition bias columns for the q|k strips
    bias_qk = const_pool.tile([128, 8], F32, tag="biasqk")
    nc.sync.dma_start(bias_qk[:], bqk.rearrange("(s p) -> p s", p=128))

    # ---- phase 2: qkv projection ----
    # Emission order: strips q0,k0 first (so head-pair 0's S^T/exp
    # stream starts as early as possible), then v (PV needs it), then
    # the remaining strips -- whose PE-dense chains serve as filler
    # while attention is paced by ACT exp.
    wnn_ctx = ExitStack()
    wnn_pool = wnn_ctx.enter_context(tc.tile_pool(name="wnn", bufs=2))
    pqk_ctx = ExitStack()
    pqk_pool = pqk_ctx.enter_context(
        tc.tile_pool(name="pqk", bufs=2, space="PSUM"))

    def qk_strip(nn):
        wn = wnn_pool.tile([128, CS_AUG, 128], BF16, tag="wnn")
        nc.sync.dma_start(
            wn[:],
            wqkv[:, nn * 128:(nn + 1) * 128]
            .rearrange("(s p) n -> p s n", p=128))
        for tb in range(TB):
            ps = pqk_pool.tile([128, 512], F32, tag="pqk")
            for s in range(CS):
                nc.tensor.matmul(ps[:], wn[:, s, :],
                                 xt[s][:, tb * 512:(tb + 1) * 512],
                                 start=(s == 0), stop=(s == CS - 1))
            # evacuate with the b_attn bias folded in (per-partition)
            nc.vector.tensor_scalar_add(
                qkt[nn][:, tb * 512:(tb + 1) * 512], ps[:],
                bias_qk[:, nn:nn + 1])

    qk_strip(0)
    qk_strip(4)

    # v_aug = x_aug @ (wqkv cols 1024..1536), natural layout
    with tc.tile_pool(name="wv", bufs=1) as wv_pool, \
         tc.tile_pool(name="pv", bufs=2, space="PSUM") as pv_pool:
        wv = wv_pool.tile([128, CS_AUG, NV], BF16, tag="wv")
        for s in range(CS_AUG):  # per-strip so the first chains start early
            nc.sync.dma_start(
                wv[:, s, :], wqkv[s * 128:(s + 1) * 128, NQK:])
        for tt in range(TT):
            ps = pv_pool.tile([128, NV], F32, tag="pv")
            for s in range(CS_AUG):
                lhsT = (ones_strip[:, 0:128] if s == CS
                        else xt[s][:, tt * 128:(tt + 1) * 128])
                nc.tensor.matmul(ps[:], lhsT, wv[:, s, :],
                                 start=(s == 0), stop=(s == CS_AUG - 1))
            nc.vector.tensor_copy(
                vau[tt][:, :, DH:],
                ps[:].rearrange("p (h d) -> p h d", d=DH))

    for nn in (1, 5, 2, 6, 3, 7):
        qk_strip(nn)
    wnn_ctx.close()
    pqk_ctx.close()
    xt_ctx.close()  # release x^T strips

    # ---- phase 3: attention + projection, i-block-outer ----
    # Per (ib, hp, jj): two row-group-concurrent K=64 S^T matmuls, one
    # Exp, diag mask-mul, two PV matmuls accumulating [ones|v] @ p.
    # After all 4 head-pairs finish i-block ib, that block's projection
    # tiles are emitted -- they serve as PE filler while the next
    # block's attention is paced by ACT exp and the psy release.
    wp_pool = ctx.enter_context(tc.tile_pool(name="wp", bufs=1))
    wp = wp_pool.tile([128, NV // 128, C], BF16, tag="wp")
    nc.sync.dma_start(wp[:], wproj.rearrange("(s p) n -> p s n", p=128))
    with tc.tile_pool(name="ptile", bufs=3) as pt_sb_pool, \
         tc.tile_pool(name="ntile", bufs=2) as n_sb_pool, \
         tc.tile_pool(name="osb", bufs=3) as osb_pool, \
         tc.tile_pool(name="ps_s", bufs=2, space="PSUM") as ps_s_pool, \
         tc.tile_pool(name="ps_y", bufs=1, space="PSUM") as ps_y_pool:
        for ib in range(TB):
            isl = slice(ib * 512, (ib + 1) * 512)
            jmax = 4 * ib + 3
            for hp in range(HL // 2):
                qs = qkt[hp]              # q strip: heads (2hp, 2hp+1)
                ks = qkt[4 + hp]          # k strip
                ps_y = [ps_y_pool.tile([128, 512], F32,
                                       tag=f"psy{u}{hp % 2}",
                                       name=f"psy{u}_{hp}_{ib}")
                        for u in range(2)]
                def s_exp(jj):
                    # S^T pair + exp + diagonal mask for one j-tile
                    off = max(0, 128 * (jj - 4 * ib))
                    ps_s = ps_s_pool.tile([128, 2, 512], F32, tag="pss")
                    for u in range(2):   # head-pair halves: base 0 / 64
                        plo = 64 * u
                        nc.tensor.matmul(
                            ps_s[:, u, off:],
                            ks[plo:plo + DH, jj * 128:(jj + 1) * 128],
                            qs[plo:plo + DH, ib * 512 + off:(ib + 1) * 512],
                            start=True, stop=True)
                    p = pt_sb_pool.tile([128, 2, 512], BF16, tag="pt")
                    nc.scalar.activation(p[:, :, off:], ps_s[:, :, off:],
                                         ACT_EXP, scale=SCALE)
                    if jj >= 4 * ib:       # diagonal tile: zero i < j
                        nc.vector.tensor_mul(
                            p[:, :, off:off + 128],
                            p[:, :, off:off + 128],
                            mask01[:, None, :].broadcast_to([128, 2, 128]))
                    return p

                def pv(jj, p):
                    off = max(0, 128 * (jj - 4 * ib))
                    for u in range(2):
                        nc.tensor.matmul(ps_y[u][:, off:],
                                         vau[jj][:, 2 * hp + u, :],
                                         p[:, u, off:],
                                         start=(jj == 0), stop=(jj == jmax))

                # software pipeline: S(jj+1) issues before PV(jj) so PE
                # has independent work while ACT finishes exp(jj)
                p_prev = s_exp(0)
                for jj in range(1, jmax + 1):
                    p_cur = s_exp(jj)
                    pv(jj - 1, p_prev)
                    p_prev = p_cur
                pv(jmax, p_prev)
                for u in range(2):
                    plo = 64 * u
                    rbb = n_sb_pool.tile([64, 512], F32, tag=f"rbb{u}")
                    nc.vector.reciprocal_approx_fast(
                        out=rbb[:], in_=ps_y[u][0:64, :])
                    nc.vector.tensor_mul(yt[hp][plo:plo + DH, isl],
                                         ps_y[u][64:128, :], rbb[:])
            # projection for this i-block (psum borrows the psy slots --
            # 8 banks total: 4 ps_s + 4 shared psy/proj)
            for tt in range(4 * ib, 4 * ib + 4):
                o_sb = osb_pool.tile([128, C], F32, tag="osb")
                for nb in range(C // 512):
                    ps = ps_y_pool.tile([128, 512], F32,
                                        tag=f"psy{nb}{tt % 2}",
                                        name=f"po{tt}_{nb}")
                    for s in range(NV // 128):
                        nc.tensor.matmul(
                            ps[:],
                            yt[s][:, tt * 128:(tt + 1) * 128],
                            wp[:, s, nb * 512:(nb + 1) * 512],
                            start=(s == 0), stop=(s == NV // 128 - 1))
                    osl = slice(nb * 512, (nb + 1) * 512)
                    if (tt + nb) % 2 == 0:
                        nc.scalar.copy(o_sb[:, osl], ps[:])
                    else:
                        nc.vector.tensor_copy(o_sb[:, osl], ps[:])
                nc.sync.dma_start(out[tt * 128:(tt + 1) * 128, :], o_sb[:])


_BUILD_LOCK = threading.Lock()
_CACHED = {}


def build_nc(repeat=1):
    with _BUILD_LOCK:
        if repeat in _CACHED:
            return _CACHED[repeat]
        nc = bacc.Bacc("TRN2", debug=False)
        x = nc.dram_tensor("x", [C, T], BF16, kind="ExternalInput").ap()
        wqkv = nc.dram_tensor("wqkv", [CS_AUG * 128, 3 * NV], BF16,
                              kind="ExternalInput").ap()
        wproj = nc.dram_tensor("wproj", [NV, C], BF16,
                               kind="ExternalInput").ap()
        bqk = nc.dram_tensor("bqk", [NQK], F32, kind="ExternalInput").ap()
        out = nc.dram_tensor("out", [T, C], F32, kind="ExternalOutput").ap()
        with tile.TileContext(nc, pool_alloc_mode="queue") as tc:
            for _ in range(repeat):
                with ExitStack() as ctx:
                    build_attention_kernel(ctx, tc, x, wqkv, wproj, bqk, out)
        nc.compile()
        _CACHED[repeat] = nc
        return nc


def shard_inputs(x, w_attn, b_attn, w_proj, b_proj):
    """Build the per-core input maps (numpy, bf16)."""
    x = np.asarray(x, dtype=np.float32)
    w_attn = np.asarray(w_attn, dtype=np.float32)
    b_attn = np.asarray(b_attn, dtype=np.float32)
    w_proj = np.asarray(w_proj, dtype=np.float32)
    in_maps = []
    for c in range(N_CORES):
        b, hh = divmod(c, 2)
        cols = np.r_[hh * 512:(hh + 1) * 512,
                     C + hh * 512:C + (hh + 1) * 512,
                     2 * C + hh * 512:2 * C + (hh + 1) * 512]
        w_aug = np.zeros((CS_AUG * 128, 3 * NV), np.float32)
        w_aug[:C] = w_attn[:, cols]
        w_aug[C] = b_attn[cols]
        in_maps.append({
            "x": np.ascontiguousarray(x[b].T).astype(NP_BF16),
            "wqkv": w_aug.astype(NP_BF16),
            "wproj": np.ascontiguousarray(
                w_proj[hh * 512:(hh + 1) * 512]).astype(NP_BF16),
            "bqk": np.ascontiguousarray(b_attn[cols[:NQK]]),
        })
    return in_maps


def kernel(x, w_attn, b_attn, w_proj, b_proj, _profile=False, _tmpdir=None):
    nc = build_nc()
    in_maps = shard_inputs(x, w_attn, b_attn, w_proj, b_proj)
    res = run_bass_kernel_spmd(nc, in_maps, list(range(N_CORES)),
                               trace=_profile, tmpdir=_tmpdir)
    b_proj = np.asarray(b_proj, dtype=np.float32)
    out = np.empty((B, T, C), np.float32)
    for b in range(B):
        out[b] = res.results[2 * b]["out"] + res.results[2 * b + 1]["out"] \
            + b_proj[None, :]
    if _profile:
        return out, res
    return out


# revision 43
# speedup vs baseline: 1.0518x; 1.0018x over previous
"""Causal multi-head attention block on 8 Trainium2 NeuronCores.

Sharding: 8 cores = 4 batches (data parallel) x 2 head-groups (tensor
parallel over heads). Core c handles batch c//2 and global heads
(c%2)*8 .. (c%2)*8+8. Each core computes a partial output projection
(split-K over its 512 head-output channels); the host sums the two
partials per batch and adds b_proj.

Per-core kernel (bf16 operands, fp32 PSUM accumulation):
  inputs:  x = x^T [1024, 2048] bf16 (host pre-transposes the batch),
           wqkv [1152, 1536] bf16 (rows 0..1023 = w_attn cols for this
           core's q|k|v heads, row 1024 = b_attn slice, rest zero),
           wproj [512, 1024] bf16
  output:  out [2048, 1024] fp32 = partial projection

Design notes (vs the fp32r baseline this evolved from):
  - x arrives pre-transposed; x^T strips are contiguous DMA loads.
  - All matmul operands are bf16: 1 cycle/row at any N (exact causal
    trimming of diagonal tiles), and FWL fast weight loads.
  - S^T tiles [j=128, head-pair, i=512] fp32 psum; one Exp per tile.
  - PV uses M=128 stationary [v_h (64 cols) | ones (64 cols)]: rows
    64..127 of the PV psum replicate the softmax denominator, so the
    reciprocal runs as one custom-DVE reciprocal_approx_fast (ACT
    Ln/Exp would thrash activation table sets; plain DVE reciprocal
    is ~6.4ns/elem/lane).
  - b_attn for the q|k strips folds into the psum evacuation as a
    per-partition tensor_scalar_add; the v strip keeps the x_aug
    ones-row augmentation.
  - qkT strips are emitted q0,k0,q1,k1,... so attention for head-pair
    0 overlaps the rest of the qkv projection.
"""

import threading
from contextlib import ExitStack

import numpy as np
import ml_dtypes

import concourse.bass as bass
import concourse.mybir as mybir
import concourse.tile as tile
from concourse import bacc
from concourse.bass_utils import run_bass_kernel_spmd

F32 = mybir.dt.float32
BF16 = mybir.dt.bfloat16
NP_BF16 = ml_dtypes.bfloat16

B, T, C = 4, 2048, 1024
H, DH = 16, 64
N_CORES = 8
HL = 8                  # local heads per core
NQK = 2 * HL * DH       # 1024 qkT rows (q 512 | k 512)
NV = HL * DH            # 512 v cols
CS = C // 128           # 8 real c-strips
CS_AUG = CS + 1         # + bias strip
TT = T // 128           # 16 token tiles
TB = T // 512           # 4 token blocks
SCALE = 1.0 / 8.0       # 1/sqrt(DH)
ACT_EXP = mybir.ActivationFunctionType.Exp


def build_attention_kernel(ctx: ExitStack, tc: tile.TileContext,
                           x: bass.AP, wqkv: bass.AP, wproj: bass.AP,
                           bqk: bass.AP, out: bass.AP):
    nc = tc.nc

    const_pool = ctx.enter_context(tc.tile_pool(name="const", bufs=1))
    # x_aug^T bias strip: row 0 ones, rows 1..127 zero.
    ones_strip = const_pool.tile([128, 512], BF16, tag="ones")
    nc.gpsimd.memset(ones_strip[:], 0.0)
    nc.gpsimd.memset(ones_strip[0:1, :], 1.0)
    # causal diag mask: 1 where i >= j (keep), 0 where i < j
    mask01 = const_pool.tile([128, 128], BF16, tag="mask01")
    nc.gpsimd.memset(mask01[:], 1.0)
    nc.gpsimd.affine_select(
        out=mask01[:], in_=mask01[:],
        compare_op=mybir.AluOpType.is_ge, fill=0.0, base=0,
        pattern=[[1, 128]], channel_multiplier=-1)

    # persistent SBUF
    qkt_pool = ctx.enter_context(tc.tile_pool(name="qkt", bufs=1))
    qkt = [qkt_pool.tile([128, T], BF16, tag=f"qkt{s}", name=f"qkt{s}")
           for s in range(NQK // 128)]
    vau_pool = ctx.enter_context(tc.tile_pool(name="vau", bufs=1))
    # [j, h, 0:64] = ones (denominator replicator; base-0 so the
    # custom-DVE reciprocal reads PSUM partitions 0..63 -- a shifted
    # base corrupts InstCustomDveAnt); [j, h, 64:128] = v_h
    vau = [vau_pool.tile([128, HL, 2 * DH], BF16, tag=f"v{tt}",
                         name=f"vau{tt}")
           for tt in range(TT)]
    for tt in range(TT):
        nc.gpsimd.memset(vau[tt][:, :, 0:DH], 1.0)
    yt_pool = ctx.enter_context(tc.tile_pool(name="yt", bufs=1))
    yt = [yt_pool.tile([128, T], BF16, tag=f"yt{s}", name=f"yt{s}")
          for s in range(NV // 128)]

    # ---- phases 1-2 share the x^T strips; freed before attention ----
    xt_ctx = ExitStack()
    xt_pool = xt_ctx.enter_context(tc.tile_pool(name="xt", bufs=1))
    xt = [xt_pool.tile([128, T], BF16, tag=f"xt{s}", name=f"xt{s}")
          for s in range(CS)]

    # ---- phase 1: x^T strips (x is pre-transposed host-side) ----
    for s in range(CS):
        nc.sync.dma_start(xt[s][:], x[s * 128:(s + 1) * 128, :])
    # b_attn per-partition bias columns for the q|k strips
    bias_qk = const_pool.tile([128, 8], F32, tag="biasqk")
    nc.sync.dma_start(bias_qk[:], bqk.rearrange("(s p) -> p s", p=128))

    # ---- phase 2: qkv projection ----
    # Emission order: strips q0,k0 first (so head-pair 0's S^T/exp
    # stream starts as early as possible), then v (PV needs it), then
    # the remaining strips -- whose PE-dense chains serve as filler
    # while attention is paced by ACT exp.
    wnn_ctx = ExitStack()
    wnn_pool = wnn_ctx.enter_context(tc.tile_pool(name="wnn", bufs=2))
    pqk_ctx = ExitStack()
    pqk_pool = pqk_ctx.enter_context(
        tc.tile_pool(name="pqk", bufs=2, space="PSUM"))

    def qk_strip(nn):
        wn = wnn_pool.tile([128, CS_AUG, 128], BF16, tag="wnn")
        nc.sync.dma_start(
            wn[:],
            wqkv[:, nn * 128:(nn + 1) * 128]
            .rearrange("(s p) n -> p s n", p=128))
        for tb in range(TB):
            ps = pqk_pool.tile([128, 512], F32, tag="pqk")
            for s in range(CS):
                nc.tensor.matmul(ps[:], wn[:, s, :],
                                 xt[s][:, tb * 512:(tb + 1) * 512],
                                 start=(s == 0), stop=(s == CS - 1))
            # evacuate with the b_attn bias folded in (per-partition)
            nc.vector.tensor_scalar_add(
                qkt[nn][:, tb * 512:(tb + 1) * 512], ps[:],
                bias_qk[:, nn:nn + 1])

    qk_strip(0)
    qk_strip(4)

    # v_aug = x_aug @ (wqkv cols 1024..1536), natural layout
    with tc.tile_pool(name="wv", bufs=1) as wv_pool, \
         tc.tile_pool(name="pv", bufs=2, space="PSUM") as pv_pool:
        wv = wv_pool.tile([128, CS_AUG, NV], BF16, tag="wv")
        for s in range(CS_AUG):  # per-strip so the first chains start early
            nc.sync.dma_start(
                wv[:, s, :], wqkv[s * 128:(s + 1) * 128, NQK:])
        for tt in range(TT):
            ps = pv_pool.tile([128, NV], F32, tag="pv")
            for s in range(CS_AUG):
                lhsT = (ones_strip[:, 0:128] if s == CS
                        else xt[s][:, tt * 128:(tt + 1) * 128])
                nc.tensor.matmul(ps[:], lhsT, wv[:, s, :],
                                 start=(s == 0), stop=(s == CS_AUG - 1))
            nc.vector.tensor_copy(
                vau[tt][:, :, DH:],
                ps[:].rearrange("p (h d) -> p h d", d=DH))

    for nn in (1, 5, 2, 6, 3, 7):
        qk_strip(nn)
    wnn_ctx.close()
    pqk_ctx.close()
    xt_ctx.close()  # release x^T strips

    # ---- phase 3: attention + projection, i-block-outer ----
    # Per (ib, hp, jj): two row-group-concurrent K=64 S^T matmuls, one
    # Exp, diag mask-mul, two PV matmuls accumulating [ones|v] @ p.
    # After all 4 head-pairs finish i-block ib, that block's projection
    # tiles are emitted -- they serve as PE filler while the next
    # block's attention is paced by ACT exp and the psy release.
    wp_pool = ctx.enter_context(tc.tile_pool(name="wp", bufs=1))
    wp = wp_pool.tile([128, NV // 128, C], BF16, tag="wp")
    nc.sync.dma_start(wp[:], wproj.rearrange("(s p) n -> p s n", p=128))
    with tc.tile_pool(name="ptile", bufs=4) as pt_sb_pool, \
         tc.tile_pool(name="ntile", bufs=2) as n_sb_pool, \
         tc.tile_pool(name="osb", bufs=3) as osb_pool, \
         tc.tile_pool(name="ps_s", bufs=3, space="PSUM") as ps_s_pool, \
         tc.tile_pool(name="ps_y", bufs=1, space="PSUM") as ps_y_pool:
        for ib in range(TB):
            isl = slice(ib * 512, (ib + 1) * 512)
            jmax = 4 * ib + 3
            for hp in range(HL // 2):
                qs = qkt[hp]              # q strip: heads (2hp, 2hp+1)
                ks = qkt[4 + hp]          # k strip
                ps_y = [ps_y_pool.tile([128, 512], F32, tag=f"psy{u}",
                                       name=f"psy{u}_{hp}_{ib}")
                        for u in range(2)]
                def s_exp(jj):
                    # S^T pair + exp + diagonal mask for one j-tile
                    off = max(0, 128 * (jj - 4 * ib))
                    ps_s = ps_s_pool.tile([128, 2, 512], F32, tag="pss")
                    for u in range(2):   # head-pair halves: base 0 / 64
                        plo = 64 * u
                        nc.tensor.matmul(
                            ps_s[:, u, off:],
                            ks[plo:plo + DH, jj * 128:(jj + 1) * 128],
                            qs[plo:plo + DH, ib * 512 + off:(ib + 1) * 512],
                            start=True, stop=True)
                    p = pt_sb_pool.tile([128, 2, 512], BF16, tag="pt")
                    nc.scalar.activation(p[:, :, off:], ps_s[:, :, off:],
                                         ACT_EXP, scale=SCALE)
                    if jj >= 4 * ib:       # diagonal tile: zero i < j
                        nc.vector.tensor_mul(
                            p[:, :, off:off + 128],
                            p[:, :, off:off + 128],
                            mask01[:, None, :].broadcast_to([128, 2, 128]))
                    return p

                def pv(jj, p):
                    off = max(0, 128 * (jj - 4 * ib))
                    for u in range(2):
                        nc.tensor.matmul(ps_y[u][:, off:],
                                         vau[jj][:, 2 * hp + u, :],
                                         p[:, u, off:],
                                         start=(jj == 0), stop=(jj == jmax))

                # software pipeline, skew 2: S(jj+2) issues before
                # PV(jj) so the PE always has independent work queued
                # while ACT works through the exp stream
                p0 = s_exp(0)
                p1 = s_exp(1)
                for jj in range(2, jmax + 1):
                    p2 = s_exp(jj)
                    pv(jj - 2, p0)
                    p0, p1 = p1, p2
                pv(jmax - 1, p0)
                pv(jmax, p1)
                for u in range(2):
                    plo = 64 * u
                    rbb = n_sb_pool.tile([64, 512], F32, tag=f"rbb{u}")
                    nc.vector.reciprocal_approx_fast(
                        out=rbb[:], in_=ps_y[u][0:64, :])
                    nc.vector.tensor_mul(yt[hp][plo:plo + DH, isl],
                                         ps_y[u][64:128, :], rbb[:])
            # projection for this i-block (psum borrows the psy slots --
            # 8 banks total: 4 ps_s + 4 shared psy/proj)
            for tt in range(4 * ib, 4 * ib + 4):
                o_sb = osb_pool.tile([128, C], F32, tag="osb")
                for nb in range(C // 512):
                    ps = ps_y_pool.tile([128, 512], F32, tag=f"psy{nb}",
                                        name=f"po{tt}_{nb}")
                    for s in range(NV // 128):
                        nc.tensor.matmul(
                            ps[:],
                            yt[s][:, tt * 128:(tt + 1) * 128],
                            wp[:, s, nb * 512:(nb + 1) * 512],
                            start=(s == 0), stop=(s == NV // 128 - 1))
                    osl = slice(nb * 512, (nb + 1) * 512)
                    nc.vector.tensor_copy(o_sb[:, osl], ps[:])
                nc.sync.dma_start(out[tt * 128:(tt + 1) * 128, :], o_sb[:])


_BUILD_LOCK = threading.Lock()
_CACHED = {}


def build_nc(repeat=1):
    with _BUILD_LOCK:
        if repeat in _CACHED:
            return _CACHED[repeat]
        nc = bacc.Bacc("TRN2", debug=False)
        x = nc.dram_tensor("x", [C, T], BF16, kind="ExternalInput").ap()
        wqkv = nc.dram_tensor("wqkv", [CS_AUG * 128, 3 * NV], BF16,
                              kind="ExternalInput").ap()
        wproj = nc.dram_tensor("wproj", [NV, C], BF16,
                               kind="ExternalInput").ap()
        bqk = nc.dram_tensor("bqk", [NQK], F32, kind="ExternalInput").ap()
        out = nc.dram_tensor("out", [T, C], F32, kind="ExternalOutput").ap()
        with tile.TileContext(nc, pool_alloc_mode="queue") as tc:
            for _ in range(repeat):
                with ExitStack() as ctx:
                    build_attention_kernel(ctx, tc, x, wqkv, wproj, bqk, out)
        nc.compile()
        _CACHED[repeat] = nc
        return nc


def shard_inputs(x, w_attn, b_attn, w_proj, b_proj):
    """Build the per-core input maps (numpy, bf16)."""
    x = np.asarray(x, dtype=np.float32)
    w_attn = np.asarray(w_attn, dtype=np.float32)
    b_attn = np.asarray(b_attn, dtype=np.float32)
    w_proj = np.asarray(w_proj, dtype=np.float32)
    in_maps = []
    for c in range(N_CORES):
        b, hh = divmod(c, 2)
        cols = np.r_[hh * 512:(hh + 1) * 512,
                     C + hh * 512:C + (hh + 1) * 512,
                     2 * C + hh * 512:2 * C + (hh + 1) * 512]
        w_aug = np.zeros((CS_AUG * 128, 3 * NV), np.float32)
        w_aug[:C] = w_attn[:, cols]
        w_aug[C] = b_attn[cols]
        in_maps.append({
            "x": np.ascontiguousarray(x[b].T).astype(NP_BF16),
            "wqkv": w_aug.astype(NP_BF16),
            "wproj": np.ascontiguousarray(
                w_proj[hh * 512:(hh + 1) * 512]).astype(NP_BF16),
            "bqk": np.ascontiguousarray(b_attn[cols[:NQK]]),
        })
    return in_maps


def kernel(x, w_attn, b_attn, w_proj, b_proj, _profile=False, _tmpdir=None):
    nc = build_nc()
    in_maps = shard_inputs(x, w_attn, b_attn, w_proj, b_proj)
    res = run_bass_kernel_spmd(nc, in_maps, list(range(N_CORES)),
                               trace=_profile, tmpdir=_tmpdir)
    b_proj = np.asarray(b_proj, dtype=np.float32)
    out = np.empty((B, T, C), np.float32)
    for b in range(B):
        out[b] = res.results[2 * b]["out"] + res.results[2 * b + 1]["out"] \
            + b_proj[None, :]
    if _profile:
        return out, res
    return out


# revision 45
# speedup vs baseline: 1.1411x; 1.0849x over previous
"""Causal multi-head attention block on 8 Trainium2 NeuronCores.

Sharding: 8 cores = 4 batches (data parallel) x 2 head-groups (tensor
parallel over heads). Core c handles batch c//2 and global heads
(c%2)*8 .. (c%2)*8+8. Each core computes a partial output projection
(split-K over its 512 head-output channels); the host sums the two
partials per batch and adds b_proj.

Per-core kernel (bf16 operands, fp32 PSUM accumulation):
  inputs:  x = x^T [1024, 2048] bf16 (host pre-transposes the batch),
           wqkv [1152, 1536] bf16 (rows 0..1023 = w_attn cols for this
           core's q|k|v heads, row 1024 = b_attn slice, rest zero),
           wproj [512, 1024] bf16
  output:  out [2048, 1024] fp32 = partial projection

Design notes (vs the fp32r baseline this evolved from):
  - x arrives pre-transposed; x^T strips are contiguous DMA loads.
  - All matmul operands are bf16: 1 cycle/row at any N (exact causal
    trimming of diagonal tiles), and FWL fast weight loads.
  - S^T tiles [j=128, head-pair, i=512] fp32 psum; one Exp per tile.
  - PV uses M=128 stationary [v_h (64 cols) | ones (64 cols)]: rows
    64..127 of the PV psum replicate the softmax denominator, so the
    reciprocal runs as one custom-DVE reciprocal_approx_fast (ACT
    Ln/Exp would thrash activation table sets; plain DVE reciprocal
    is ~6.4ns/elem/lane).
  - b_attn for the q|k strips folds into the psum evacuation as a
    per-partition tensor_scalar_add; the v strip keeps the x_aug
    ones-row augmentation.
  - qkT strips are emitted q0,k0,q1,k1,... so attention for head-pair
    0 overlaps the rest of the qkv projection.
"""

import threading
from contextlib import ExitStack

import numpy as np
import ml_dtypes

import concourse.bass as bass
import concourse.mybir as mybir
import concourse.tile as tile
from concourse import bacc
from concourse.bass_utils import run_bass_kernel_spmd

F32 = mybir.dt.float32
BF16 = mybir.dt.bfloat16
NP_BF16 = ml_dtypes.bfloat16

B, T, C = 4, 2048, 1024
H, DH = 16, 64
N_CORES = 8
HL = 8                  # local heads per core
NQK = 2 * HL * DH       # 1024 qkT rows (q 512 | k 512)
NV = HL * DH            # 512 v cols
CS = C // 128           # 8 real c-strips
CS_AUG = CS + 1         # + bias strip
TT = T // 128           # 16 token tiles
TB = T // 512           # 4 token blocks
SCALE = 1.0 / 8.0       # 1/sqrt(DH)
ACT_EXP = mybir.ActivationFunctionType.Exp


def build_attention_kernel(ctx: ExitStack, tc: tile.TileContext,
                           x: bass.AP, wqkv: bass.AP, wproj: bass.AP,
                           bqk: bass.AP, out: bass.AP):
    nc = tc.nc

    const_pool = ctx.enter_context(tc.tile_pool(name="const", bufs=1))
    # x_aug^T bias strip: row 0 ones, rows 1..127 zero.
    ones_strip = const_pool.tile([128, 512], BF16, tag="ones")
    nc.gpsimd.memset(ones_strip[:], 0.0)
    nc.gpsimd.memset(ones_strip[0:1, :], 1.0)
    # causal diag mask: 1 where i >= j (keep), 0 where i < j
    mask01 = const_pool.tile([128, 128], BF16, tag="mask01")
    nc.gpsimd.memset(mask01[:], 1.0)
    nc.gpsimd.affine_select(
        out=mask01[:], in_=mask01[:],
        compare_op=mybir.AluOpType.is_ge, fill=0.0, base=0,
        pattern=[[1, 128]], channel_multiplier=-1)

    # persistent SBUF
    qkt_pool = ctx.enter_context(tc.tile_pool(name="qkt", bufs=1))
    qkt = [qkt_pool.tile([128, T], BF16, tag=f"qkt{s}", name=f"qkt{s}")
           for s in range(NQK // 128)]
    vau_pool = ctx.enter_context(tc.tile_pool(name="vau", bufs=1))
    # [j, h, 0:64] = ones (denominator replicator; base-0 so the
    # custom-DVE reciprocal reads PSUM partitions 0..63 -- a shifted
    # base corrupts InstCustomDveAnt); [j, h, 64:128] = v_h
    vau = [vau_pool.tile([128, HL, 2 * DH], BF16, tag=f"v{tt}",
                         name=f"vau{tt}")
           for tt in range(TT)]
    for tt in range(TT):
        nc.gpsimd.memset(vau[tt][:, :, 0:DH], 1.0)
    yt_pool = ctx.enter_context(tc.tile_pool(name="yt", bufs=1))
    yt = [yt_pool.tile([128, T], BF16, tag=f"yt{s}", name=f"yt{s}")
          for s in range(NV // 128)]

    # x^T strips stay resident (late qkv filler chains still read them)
    xt_pool = ctx.enter_context(tc.tile_pool(name="xt", bufs=1))
    xt = [xt_pool.tile([128, T], BF16, tag=f"xt{s}", name=f"xt{s}")
          for s in range(CS)]

    # ---- phase 1: x^T strips (x is pre-transposed host-side) ----
    for s in range(CS):
        nc.sync.dma_start(xt[s][:], x[s * 128:(s + 1) * 128, :])
    # b_attn per-partition bias columns for the q|k strips
    bias_qk = const_pool.tile([128, 8], F32, tag="biasqk")
    nc.sync.dma_start(bias_qk[:], bqk.rearrange("(s p) -> p s", p=128))

    # ---- phases 2+3: qkv / attention / proj, filler-interleaved ----
    # PE executes in emission order, so the PE-dense qkv chains and the
    # ACT-paced attention must be woven together: attention runs
    # hp-outer (head-pair hp needs only strips hp, 4+hp), and a filler
    # queue drips one qkv chain / vau tile / proj chunk between
    # attention j-tiles. All chain psums share one 3-slot pool; the
    # psy pool (2 slots) doubles as proj psum. 8 banks total.
    wp_pool = ctx.enter_context(tc.tile_pool(name="wp", bufs=1))
    wp = wp_pool.tile([128, NV // 128, C], BF16, tag="wp")
    nc.sync.dma_start(wp[:], wproj.rearrange("(s p) n -> p s n", p=128))
    wv_pool = ctx.enter_context(tc.tile_pool(name="wv", bufs=1))
    wv = wv_pool.tile([128, CS_AUG, NV], BF16, tag="wv")
    for s in range(CS_AUG):
        nc.sync.dma_start(wv[:, s, :], wqkv[s * 128:(s + 1) * 128, NQK:])
    wnn_pool = ctx.enter_context(tc.tile_pool(name="wnn", bufs=1))
    wn = [wnn_pool.tile([128, CS_AUG, 128], BF16, tag=f"wnn{nn}",
                        name=f"wnn{nn}")
          for nn in range(8)]
    for nn in range(8):
        nc.sync.dma_start(
            wn[nn][:],
            wqkv[:, nn * 128:(nn + 1) * 128]
            .rearrange("(s p) n -> p s n", p=128))

    pt_sb_pool = ctx.enter_context(tc.tile_pool(name="ptile", bufs=4))
    n_sb_pool = ctx.enter_context(tc.tile_pool(name="ntile", bufs=2))
    osb_pool = ctx.enter_context(tc.tile_pool(name="osb", bufs=2))
    ps_s_pool = ctx.enter_context(
        tc.tile_pool(name="ps_s", bufs=3, space="PSUM"))
    ps_y_pool = ctx.enter_context(
        tc.tile_pool(name="ps_y", bufs=1, space="PSUM"))

    def qk_chain(nn, tb):
        # one [128, 512] block of qkT strip nn (borrows a pss slot)
        ps = ps_s_pool.tile([128, 2, 512], F32, tag="pss",
                            name=f"pqk{nn}_{tb}")
        for s in range(CS):
            nc.tensor.matmul(ps[:, 0, :], wn[nn][:, s, :],
                             xt[s][:, tb * 512:(tb + 1) * 512],
                             start=(s == 0), stop=(s == CS - 1))
        # evacuate with the b_attn bias folded in (per-partition)
        nc.vector.tensor_scalar_add(
            qkt[nn][:, tb * 512:(tb + 1) * 512], ps[:, 0, :],
            bias_qk[:, nn:nn + 1])

    def vau_tile(tt):
        ps = ps_s_pool.tile([128, 2, 512], F32, tag="pss",
                            name=f"pv{tt}")
        for s in range(CS_AUG):
            lhsT = (ones_strip[:, 0:128] if s == CS
                    else xt[s][:, tt * 128:(tt + 1) * 128])
            nc.tensor.matmul(ps[:, 0, :], lhsT, wv[:, s, :],
                             start=(s == 0), stop=(s == CS_AUG - 1))
        nc.vector.tensor_copy(
            vau[tt][:, :, DH:],
            ps[:, 0, :].rearrange("p (h d) -> p h d", d=DH))

    osb = [osb_pool.tile([128, C], F32, tag=f"osb{i}", name=f"osb{i}")
           for i in range(2)]

    def proj_chunk(tt, nb):
        ps = ps_s_pool.tile([128, 2, 512], F32, tag="pss",
                            name=f"po{tt}_{nb}")
        for s in range(NV // 128):
            nc.tensor.matmul(
                ps[:, 0, :],
                yt[s][:, tt * 128:(tt + 1) * 128],
                wp[:, s, nb * 512:(nb + 1) * 512],
                start=(s == 0), stop=(s == NV // 128 - 1))
        o_sb = osb[tt % 2]
        nc.vector.tensor_copy(o_sb[:, nb * 512:(nb + 1) * 512],
                              ps[:, 0, :])
        if nb == C // 512 - 1:
            nc.sync.dma_start(out[tt * 128:(tt + 1) * 128, :], o_sb[:])

    def attn_gen(ib, hp):
        # generator: yields after each j-tile so fillers can interleave
        isl = slice(ib * 512, (ib + 1) * 512)
        jmax = 4 * ib + 3
        qs = qkt[hp]              # q strip: heads (2hp, 2hp+1)
        ks = qkt[4 + hp]          # k strip
        ps_y = [ps_y_pool.tile([128, 512], F32, tag=f"psy{u}",
                               name=f"psy{u}_{hp}_{ib}")
                for u in range(2)]

        def s_exp(jj):
            off = max(0, 128 * (jj - 4 * ib))
            ps_s = ps_s_pool.tile([128, 2, 512], F32, tag="pss")
            for u in range(2):   # head-pair halves: base 0 / 64
                plo = 64 * u
                nc.tensor.matmul(
                    ps_s[:, u, off:],
                    ks[plo:plo + DH, jj * 128:(jj + 1) * 128],
                    qs[plo:plo + DH, ib * 512 + off:(ib + 1) * 512],
                    start=True, stop=True)
            p = pt_sb_pool.tile([128, 2, 512], BF16, tag="pt")
            nc.scalar.activation(p[:, :, off:], ps_s[:, :, off:],
                                 ACT_EXP, scale=SCALE)
            if jj >= 4 * ib:       # diagonal tile: zero i < j
                nc.vector.tensor_mul(
                    p[:, :, off:off + 128],
                    p[:, :, off:off + 128],
                    mask01[:, None, :].broadcast_to([128, 2, 128]))
            return p

        def pv(jj, p):
            off = max(0, 128 * (jj - 4 * ib))
            for u in range(2):
                nc.tensor.matmul(ps_y[u][:, off:],
                                 vau[jj][:, 2 * hp + u, :],
                                 p[:, u, off:],
                                 start=(jj == 0), stop=(jj == jmax))

        # software pipeline, skew 2: S(jj+2) issues before PV(jj)
        p0 = s_exp(0)
        yield
        p1 = s_exp(1)
        yield
        for jj in range(2, jmax + 1):
            p2 = s_exp(jj)
            pv(jj - 2, p0)
            p0, p1 = p1, p2
            yield
        pv(jmax - 1, p0)
        pv(jmax, p1)
        for u in range(2):
            plo = 64 * u
            rbb = n_sb_pool.tile([64, 512], F32, tag=f"rbb{u}")
            nc.vector.reciprocal_approx_fast(
                out=rbb[:], in_=ps_y[u][0:64, :])
            nc.vector.tensor_mul(yt[hp][plo:plo + DH, isl],
                                 ps_y[u][64:128, :], rbb[:])

    # lead-in: strips for hp=0 + the first vau tiles (ACT idle anyway)
    for tb in range(TB):
        qk_chain(0, tb)
    for tb in range(TB):
        qk_chain(4, tb)
    for tt in range(0, 4):
        vau_tile(tt)

    # filler queue, ordered by when attention first needs each item
    fillers = []
    for tt in range(4, TT):
        fillers.append(lambda tt=tt: vau_tile(tt))
    for hp_next in (1, 2, 3):
        for nn in (hp_next, 4 + hp_next):
            for tb in range(TB):
                fillers.append(lambda nn=nn, tb=tb: qk_chain(nn, tb))

    fi = 0
    tick = 0
    for hp in range(HL // 2):
        for ib in range(TB):
            for _ in attn_gen(ib, hp):
                tick += 1
                if tick % 2 == 0 and fi < len(fillers):
                    fillers[fi]()
                    fi += 1
            if hp == HL // 2 - 1:
                # this i-block's projection unlocks once hp3 finishes it
                for tt in range(4 * ib, 4 * ib + 4):
                    for nb in range(C // 512):
                        fillers.append(
                            lambda tt=tt, nb=nb: proj_chunk(tt, nb))
    while fi < len(fillers):   # drain: remaining proj chunks
        fillers[fi]()
        fi += 1

_BUILD_LOCK = threading.Lock()
_CACHED = {}


def build_nc(repeat=1):
    with _BUILD_LOCK:
        if repeat in _CACHED:
            return _CACHED[repeat]
        nc = bacc.Bacc("TRN2", debug=False)
        x = nc.dram_tensor("x", [C, T], BF16, kind="ExternalInput").ap()
        wqkv = nc.dram_tensor("wqkv", [CS_AUG * 128, 3 * NV], BF16,
                              kind="ExternalInput").ap()
        wproj = nc.dram_tensor("wproj", [NV, C], BF16,
                               kind="ExternalInput").ap()
        bqk = nc.dram_tensor("bqk", [NQK], F32, kind="ExternalInput").ap()
        out = nc.dram_tensor("out", [T, C], F32, kind="ExternalOutput").ap()
        with tile.TileContext(nc, pool_alloc_mode="queue") as tc:
            for _ in range(repeat):
                with ExitStack() as ctx:
                    build_attention_kernel(ctx, tc, x, wqkv, wproj, bqk, out)
        nc.compile()
        _CACHED[repeat] = nc
        return nc


def shard_inputs(x, w_attn, b_attn, w_proj, b_proj):
    """Build the per-core input maps (numpy, bf16)."""
    x = np.asarray(x, dtype=np.float32)
    w_attn = np.asarray(w_attn, dtype=np.float32)
    b_attn = np.asarray(b_attn, dtype=np.float32)
    w_proj = np.asarray(w_proj, dtype=np.float32)
    in_maps = []
    for c in range(N_CORES):
        b, hh = divmod(c, 2)
        cols = np.r_[hh * 512:(hh + 1) * 512,
                     C + hh * 512:C + (hh + 1) * 512,
                     2 * C + hh * 512:2 * C + (hh + 1) * 512]
        w_aug = np.zeros((CS_AUG * 128, 3 * NV), np.float32)
        w_aug[:C] = w_attn[:, cols]
        w_aug[C] = b_attn[cols]
        in_maps.append({
            "x": np.ascontiguousarray(x[b].T).astype(NP_BF16),
            "wqkv": w_aug.astype(NP_BF16),
            "wproj": np.ascontiguousarray(
                w_proj[hh * 512:(hh + 1) * 512]).astype(NP_BF16),
            "bqk": np.ascontiguousarray(b_attn[cols[:NQK]]),
        })
    return in_maps


def kernel(x, w_attn, b_attn, w_proj, b_proj, _profile=False, _tmpdir=None):
    nc = build_nc()
    in_maps = shard_inputs(x, w_attn, b_attn, w_proj, b_proj)
    res = run_bass_kernel_spmd(nc, in_maps, list(range(N_CORES)),
                               trace=_profile, tmpdir=_tmpdir)
    b_proj = np.asarray(b_proj, dtype=np.float32)
    out = np.empty((B, T, C), np.float32)
    for b in range(B):
        out[b] = res.results[2 * b]["out"] + res.results[2 * b + 1]["out"] \
            + b_proj[None, :]
    if _profile:
        return out, res
    return out


# revision 46
# speedup vs baseline: 1.1597x; 1.0163x over previous
"""Causal multi-head attention block on 8 Trainium2 NeuronCores.

Sharding: 8 cores = 4 batches (data parallel) x 2 head-groups (tensor
parallel over heads). Core c handles batch c//2 and global heads
(c%2)*8 .. (c%2)*8+8. Each core computes a partial output projection
(split-K over its 512 head-output channels); the host sums the two
partials per batch and adds b_proj.

Per-core kernel (bf16 operands, fp32 PSUM accumulation):
  inputs:  x = x^T [1024, 2048] bf16 (host pre-transposes the batch),
           wqkv [1152, 1536] bf16 (rows 0..1023 = w_attn cols for this
           core's q|k|v heads, row 1024 = b_attn slice, rest zero),
           wproj [512, 1024] bf16
  output:  out [2048, 1024] fp32 = partial projection

Design notes (vs the fp32r baseline this evolved from):
  - x arrives pre-transposed; x^T strips are contiguous DMA loads.
  - All matmul operands are bf16: 1 cycle/row at any N (exact causal
    trimming of diagonal tiles), and FWL fast weight loads.
  - S^T tiles [j=128, head-pair, i=512] fp32 psum; one Exp per tile.
  - PV uses M=128 stationary [v_h (64 cols) | ones (64 cols)]: rows
    64..127 of the PV psum replicate the softmax denominator, so the
    reciprocal runs as one custom-DVE reciprocal_approx_fast (ACT
    Ln/Exp would thrash activation table sets; plain DVE reciprocal
    is ~6.4ns/elem/lane).
  - b_attn for the q|k strips folds into the psum evacuation as a
    per-partition tensor_scalar_add; the v strip keeps the x_aug
    ones-row augmentation.
  - qkT strips are emitted q0,k0,q1,k1,... so attention for head-pair
    0 overlaps the rest of the qkv projection.
"""

import threading
from contextlib import ExitStack

import numpy as np
import ml_dtypes

import concourse.bass as bass
import concourse.mybir as mybir
import concourse.tile as tile
from concourse import bacc
from concourse.bass_utils import run_bass_kernel_spmd

F32 = mybir.dt.float32
BF16 = mybir.dt.bfloat16
NP_BF16 = ml_dtypes.bfloat16

B, T, C = 4, 2048, 1024
H, DH = 16, 64
N_CORES = 8
HL = 8                  # local heads per core
NQK = 2 * HL * DH       # 1024 qkT rows (q 512 | k 512)
NV = HL * DH            # 512 v cols
CS = C // 128           # 8 real c-strips
CS_AUG = CS + 1         # + bias strip
TT = T // 128           # 16 token tiles
TB = T // 512           # 4 token blocks
SCALE = 1.0 / 8.0       # 1/sqrt(DH)
ACT_EXP = mybir.ActivationFunctionType.Exp


def build_attention_kernel(ctx: ExitStack, tc: tile.TileContext,
                           x: bass.AP, wqkv: bass.AP, wproj: bass.AP,
                           bqk: bass.AP, out: bass.AP):
    nc = tc.nc

    const_pool = ctx.enter_context(tc.tile_pool(name="const", bufs=1))
    # x_aug^T bias strip: row 0 ones, rows 1..127 zero.
    ones_strip = const_pool.tile([128, 512], BF16, tag="ones")
    nc.gpsimd.memset(ones_strip[:], 0.0)
    nc.gpsimd.memset(ones_strip[0:1, :], 1.0)
    # causal diag mask: 1 where i >= j (keep), 0 where i < j
    mask01 = const_pool.tile([128, 128], BF16, tag="mask01")
    nc.gpsimd.memset(mask01[:], 1.0)
    nc.gpsimd.affine_select(
        out=mask01[:], in_=mask01[:],
        compare_op=mybir.AluOpType.is_ge, fill=0.0, base=0,
        pattern=[[1, 128]], channel_multiplier=-1)

    # persistent SBUF
    qkt_pool = ctx.enter_context(tc.tile_pool(name="qkt", bufs=1))
    qkt = [qkt_pool.tile([128, T], BF16, tag=f"qkt{s}", name=f"qkt{s}")
           for s in range(NQK // 128)]
    vau_pool = ctx.enter_context(tc.tile_pool(name="vau", bufs=1))
    # [j, h, 0:64] = ones (denominator replicator; base-0 so the
    # custom-DVE reciprocal reads PSUM partitions 0..63 -- a shifted
    # base corrupts InstCustomDveAnt); [j, h, 64:128] = v_h
    vau = [vau_pool.tile([128, HL, 2 * DH], BF16, tag=f"v{tt}",
                         name=f"vau{tt}")
           for tt in range(TT)]
    for tt in range(TT):
        nc.gpsimd.memset(vau[tt][:, :, 0:DH], 1.0)
    yt_pool = ctx.enter_context(tc.tile_pool(name="yt", bufs=1))
    yt = [yt_pool.tile([128, T], BF16, tag=f"yt{s}", name=f"yt{s}")
          for s in range(NV // 128)]

    # x^T strips stay resident (late qkv filler chains still read them)
    xt_pool = ctx.enter_context(tc.tile_pool(name="xt", bufs=1))
    xt = [xt_pool.tile([128, T], BF16, tag=f"xt{s}", name=f"xt{s}")
          for s in range(CS)]

    # ---- phase 1: x^T strips (x is pre-transposed host-side) ----
    for s in range(CS):
        nc.sync.dma_start(xt[s][:], x[s * 128:(s + 1) * 128, :])
    # b_attn per-partition bias columns for the q|k strips
    bias_qk = const_pool.tile([128, 8], F32, tag="biasqk")
    nc.sync.dma_start(bias_qk[:], bqk.rearrange("(s p) -> p s", p=128))

    # ---- phases 2+3: qkv / attention / proj, filler-interleaved ----
    # PE executes in emission order, so the PE-dense qkv chains and the
    # ACT-paced attention must be woven together: attention runs
    # hp-outer (head-pair hp needs only strips hp, 4+hp), and a filler
    # queue drips one qkv chain / vau tile / proj chunk between
    # attention j-tiles. All chain psums share one 3-slot pool; the
    # psy pool (2 slots) doubles as proj psum. 8 banks total.
    wnn_pool = ctx.enter_context(tc.tile_pool(name="wnn", bufs=1))
    wn = [wnn_pool.tile([128, CS_AUG, 128], BF16, tag=f"wnn{nn}",
                        name=f"wnn{nn}")
          for nn in range(8)]
    wv_pool = ctx.enter_context(tc.tile_pool(name="wv", bufs=1))
    wv = wv_pool.tile([128, CS_AUG, NV], BF16, tag="wv")
    # DMA priority: the first chains need wn0, wn4 and wv; wp is only
    # needed ~200us in
    for nn in (0, 4):
        nc.sync.dma_start(
            wn[nn][:],
            wqkv[:, nn * 128:(nn + 1) * 128]
            .rearrange("(s p) n -> p s n", p=128))
    for s in range(CS_AUG):
        nc.sync.dma_start(wv[:, s, :], wqkv[s * 128:(s + 1) * 128, NQK:])
    for nn in (1, 5, 2, 6, 3, 7):
        nc.sync.dma_start(
            wn[nn][:],
            wqkv[:, nn * 128:(nn + 1) * 128]
            .rearrange("(s p) n -> p s n", p=128))
    wp_pool = ctx.enter_context(tc.tile_pool(name="wp", bufs=1))
    wp = wp_pool.tile([128, NV // 128, C], BF16, tag="wp")
    nc.sync.dma_start(wp[:], wproj.rearrange("(s p) n -> p s n", p=128))

    pt_sb_pool = ctx.enter_context(tc.tile_pool(name="ptile", bufs=4))
    n_sb_pool = ctx.enter_context(tc.tile_pool(name="ntile", bufs=2))
    osb_pool = ctx.enter_context(tc.tile_pool(name="osb", bufs=2))
    ps_s_pool = ctx.enter_context(
        tc.tile_pool(name="ps_s", bufs=3, space="PSUM"))
    ps_y_pool = ctx.enter_context(
        tc.tile_pool(name="ps_y", bufs=1, space="PSUM"))

    def qk_chain(nn, tb):
        # one [128, 512] block of qkT strip nn (borrows a pss slot)
        ps = ps_s_pool.tile([128, 2, 512], F32, tag="pss",
                            name=f"pqk{nn}_{tb}")
        for s in range(CS):
            nc.tensor.matmul(ps[:, 0, :], wn[nn][:, s, :],
                             xt[s][:, tb * 512:(tb + 1) * 512],
                             start=(s == 0), stop=(s == CS - 1))
        # evacuate with the b_attn bias folded in (per-partition)
        nc.vector.tensor_scalar_add(
            qkt[nn][:, tb * 512:(tb + 1) * 512], ps[:, 0, :],
            bias_qk[:, nn:nn + 1])

    def vau_tile(tt):
        ps = ps_s_pool.tile([128, 2, 512], F32, tag="pss",
                            name=f"pv{tt}")
        for s in range(CS_AUG):
            lhsT = (ones_strip[:, 0:128] if s == CS
                    else xt[s][:, tt * 128:(tt + 1) * 128])
            nc.tensor.matmul(ps[:, 0, :], lhsT, wv[:, s, :],
                             start=(s == 0), stop=(s == CS_AUG - 1))
        nc.vector.tensor_copy(
            vau[tt][:, :, DH:],
            ps[:, 0, :].rearrange("p (h d) -> p h d", d=DH))

    osb = [osb_pool.tile([128, C], F32, tag=f"osb{i}", name=f"osb{i}")
           for i in range(2)]

    def proj_chunk(tt, nb):
        ps = ps_s_pool.tile([128, 2, 512], F32, tag="pss",
                            name=f"po{tt}_{nb}")
        for s in range(NV // 128):
            nc.tensor.matmul(
                ps[:, 0, :],
                yt[s][:, tt * 128:(tt + 1) * 128],
                wp[:, s, nb * 512:(nb + 1) * 512],
                start=(s == 0), stop=(s == NV // 128 - 1))
        o_sb = osb[tt % 2]
        nc.vector.tensor_copy(o_sb[:, nb * 512:(nb + 1) * 512],
                              ps[:, 0, :])
        if nb == C // 512 - 1:
            nc.sync.dma_start(out[tt * 128:(tt + 1) * 128, :], o_sb[:])

    def attn_gen(ib, hp):
        # generator: yields after each j-tile so fillers can interleave
        isl = slice(ib * 512, (ib + 1) * 512)
        jmax = 4 * ib + 3
        qs = qkt[hp]              # q strip: heads (2hp, 2hp+1)
        ks = qkt[4 + hp]          # k strip
        ps_y = [ps_y_pool.tile([128, 512], F32, tag=f"psy{u}",
                               name=f"psy{u}_{hp}_{ib}")
                for u in range(2)]

        def s_exp(jj):
            off = max(0, 128 * (jj - 4 * ib))
            ps_s = ps_s_pool.tile([128, 2, 512], F32, tag="pss")
            for u in range(2):   # head-pair halves: base 0 / 64
                plo = 64 * u
                nc.tensor.matmul(
                    ps_s[:, u, off:],
                    ks[plo:plo + DH, jj * 128:(jj + 1) * 128],
                    qs[plo:plo + DH, ib * 512 + off:(ib + 1) * 512],
                    start=True, stop=True)
            p = pt_sb_pool.tile([128, 2, 512], BF16, tag="pt")
            nc.scalar.activation(p[:, :, off:], ps_s[:, :, off:],
                                 ACT_EXP, scale=SCALE)
            if jj >= 4 * ib:       # diagonal tile: zero i < j
                nc.vector.tensor_mul(
                    p[:, :, off:off + 128],
                    p[:, :, off:off + 128],
                    mask01[:, None, :].broadcast_to([128, 2, 128]))
            return p

        def pv(jj, p):
            off = max(0, 128 * (jj - 4 * ib))
            for u in range(2):
                nc.tensor.matmul(ps_y[u][:, off:],
                                 vau[jj][:, 2 * hp + u, :],
                                 p[:, u, off:],
                                 start=(jj == 0), stop=(jj == jmax))

        # software pipeline, skew 2: S(jj+2) issues before PV(jj)
        p0 = s_exp(0)
        yield
        p1 = s_exp(1)
        yield
        for jj in range(2, jmax + 1):
            p2 = s_exp(jj)
            pv(jj - 2, p0)
            p0, p1 = p1, p2
            yield
        pv(jmax - 1, p0)
        pv(jmax, p1)
        for u in range(2):
            plo = 64 * u
            rbb = n_sb_pool.tile([64, 512], F32, tag=f"rbb{u}")
            nc.vector.reciprocal_approx_fast(
                out=rbb[:], in_=ps_y[u][0:64, :])
            nc.vector.tensor_mul(yt[hp][plo:plo + DH, isl],
                                 ps_y[u][64:128, :], rbb[:])

    # lead-in: strips for hp=0 + the first vau tiles (ACT idle anyway)
    for tb in range(TB):
        qk_chain(0, tb)
    for tb in range(TB):
        qk_chain(4, tb)
    for tt in range(0, 4):
        vau_tile(tt)

    # filler queue, ordered by when attention first needs each item
    fillers = []
    for tt in range(4, TT):
        fillers.append(lambda tt=tt: vau_tile(tt))
    for hp_next in (1, 2, 3):
        for nn in (hp_next, 4 + hp_next):
            for tb in range(TB):
                fillers.append(lambda nn=nn, tb=tb: qk_chain(nn, tb))

    fi = 0
    tick = 0
    for hp in range(HL // 2):
        for ib in range(TB):
            rate = 1 if hp == HL // 2 - 1 else 2
            for _ in attn_gen(ib, hp):
                tick += 1
                if tick % rate == 0 and fi < len(fillers):
                    fillers[fi]()
                    fi += 1
            if hp == HL // 2 - 1:
                # this i-block's projection unlocks once hp3 finishes it
                for tt in range(4 * ib, 4 * ib + 4):
                    for nb in range(C // 512):
                        fillers.append(
                            lambda tt=tt, nb=nb: proj_chunk(tt, nb))
    while fi < len(fillers):   # drain: remaining proj chunks
        fillers[fi]()
        fi += 1

_BUILD_LOCK = threading.Lock()
_CACHED = {}


def build_nc(repeat=1):
    with _BUILD_LOCK:
        if repeat in _CACHED:
            return _CACHED[repeat]
        nc = bacc.Bacc("TRN2", debug=False)
        x = nc.dram_tensor("x", [C, T], BF16, kind="ExternalInput").ap()
        wqkv = nc.dram_tensor("wqkv", [CS_AUG * 128, 3 * NV], BF16,
                              kind="ExternalInput").ap()
        wproj = nc.dram_tensor("wproj", [NV, C], BF16,
                               kind="ExternalInput").ap()
        bqk = nc.dram_tensor("bqk", [NQK], F32, kind="ExternalInput").ap()
        out = nc.dram_tensor("out", [T, C], F32, kind="ExternalOutput").ap()
        with tile.TileContext(nc, pool_alloc_mode="queue") as tc:
            for _ in range(repeat):
                with ExitStack() as ctx:
                    build_attention_kernel(ctx, tc, x, wqkv, wproj, bqk, out)
        nc.compile()
        _CACHED[repeat] = nc
        return nc


def shard_inputs(x, w_attn, b_attn, w_proj, b_proj):
    """Build the per-core input maps (numpy, bf16)."""
    x = np.asarray(x, dtype=np.float32)
    w_attn = np.asarray(w_attn, dtype=np.float32)
    b_attn = np.asarray(b_attn, dtype=np.float32)
    w_proj = np.asarray(w_proj, dtype=np.float32)
    in_maps = []
    for c in range(N_CORES):
        b, hh = divmod(c, 2)
        cols = np.r_[hh * 512:(hh + 1) * 512,
                     C + hh * 512:C + (hh + 1) * 512,
                     2 * C + hh * 512:2 * C + (hh + 1) * 512]
        w_aug = np.zeros((CS_AUG * 128, 3 * NV), np.float32)
        w_aug[:C] = w_attn[:, cols]
        w_aug[C] = b_attn[cols]
        in_maps.append({
            "x": np.ascontiguousarray(x[b].T).astype(NP_BF16),
            "wqkv": w_aug.astype(NP_BF16),
            "wproj": np.ascontiguousarray(
                w_proj[hh * 512:(hh + 1) * 512]).astype(NP_BF16),
            "bqk": np.ascontiguousarray(b_attn[cols[:NQK]]),
        })
    return in_maps


def kernel(x, w_attn, b_attn, w_proj, b_proj, _profile=False, _tmpdir=None):
    nc = build_nc()
    in_maps = shard_inputs(x, w_attn, b_attn, w_proj, b_proj)
    res = run_bass_kernel_spmd(nc, in_maps, list(range(N_CORES)),
                               trace=_profile, tmpdir=_tmpdir)
    b_proj = np.asarray(b_proj, dtype=np.float32)
    out = np.empty((B, T, C), np.float32)
    for b in range(B):
        out[b] = res.results[2 * b]["out"] + res.results[2 * b + 1]["out"] \
            + b_proj[None, :]
    if _profile:
        return out, res
    return out


# revision 47
# speedup vs baseline: 1.1633x; 1.0031x over previous
"""Causal multi-head attention block on 8 Trainium2 NeuronCores.

Sharding: 8 cores = 4 batches (data parallel) x 2 head-groups (tensor
parallel over heads). Core c handles batch c//2 and global heads
(c%2)*8 .. (c%2)*8+8. Each core computes a partial output projection
(split-K over its 512 head-output channels); the host sums the two
partials per batch and adds b_proj.

Per-core kernel (bf16 operands, fp32 PSUM accumulation):
  inputs:  x = x^T [1024, 2048] bf16 (host pre-transposes the batch),
           wqkv [1152, 1536] bf16 (rows 0..1023 = w_attn cols for this
           core's q|k|v heads, row 1024 = b_attn slice, rest zero),
           wproj [512, 1024] bf16
  output:  out [2048, 1024] fp32 = partial projection

Design notes (vs the fp32r baseline this evolved from):
  - x arrives pre-transposed; x^T strips are contiguous DMA loads.
  - All matmul operands are bf16: 1 cycle/row at any N (exact causal
    trimming of diagonal tiles), and FWL fast weight loads.
  - S^T tiles [j=128, head-pair, i=512] fp32 psum; one Exp per tile.
  - PV uses M=128 stationary [v_h (64 cols) | ones (64 cols)]: rows
    64..127 of the PV psum replicate the softmax denominator, so the
    reciprocal runs as one custom-DVE reciprocal_approx_fast (ACT
    Ln/Exp would thrash activation table sets; plain DVE reciprocal
    is ~6.4ns/elem/lane).
  - b_attn for the q|k strips folds into the psum evacuation as a
    per-partition tensor_scalar_add; the v strip keeps the x_aug
    ones-row augmentation.
  - qkT strips are emitted q0,k0,q1,k1,... so attention for head-pair
    0 overlaps the rest of the qkv projection.
"""

import threading
from contextlib import ExitStack

import numpy as np
import ml_dtypes

import concourse.bass as bass
import concourse.mybir as mybir
import concourse.tile as tile
from concourse import bacc
from concourse.bass_utils import run_bass_kernel_spmd

F32 = mybir.dt.float32
BF16 = mybir.dt.bfloat16
NP_BF16 = ml_dtypes.bfloat16

B, T, C = 4, 2048, 1024
H, DH = 16, 64
N_CORES = 8
HL = 8                  # local heads per core
NQK = 2 * HL * DH       # 1024 qkT rows (q 512 | k 512)
NV = HL * DH            # 512 v cols
CS = C // 128           # 8 real c-strips
CS_AUG = CS + 1         # + bias strip
TT = T // 128           # 16 token tiles
TB = T // 512           # 4 token blocks
SCALE = 1.0 / 8.0       # 1/sqrt(DH)
ACT_EXP = mybir.ActivationFunctionType.Exp


def build_attention_kernel(ctx: ExitStack, tc: tile.TileContext,
                           x: bass.AP, wqkv: bass.AP, wproj: bass.AP,
                           bqk: bass.AP, out: bass.AP):
    nc = tc.nc

    const_pool = ctx.enter_context(tc.tile_pool(name="const", bufs=1))
    # x_aug^T bias strip: row 0 ones, rows 1..127 zero.
    ones_strip = const_pool.tile([128, 512], BF16, tag="ones")
    nc.gpsimd.memset(ones_strip[:], 0.0)
    nc.gpsimd.memset(ones_strip[0:1, :], 1.0)
    # causal diag mask: 1 where i >= j (keep), 0 where i < j
    mask01 = const_pool.tile([128, 128], BF16, tag="mask01")
    nc.gpsimd.memset(mask01[:], 1.0)
    nc.gpsimd.affine_select(
        out=mask01[:], in_=mask01[:],
        compare_op=mybir.AluOpType.is_ge, fill=0.0, base=0,
        pattern=[[1, 128]], channel_multiplier=-1)

    # persistent SBUF
    qkt_pool = ctx.enter_context(tc.tile_pool(name="qkt", bufs=1))
    qkt = [qkt_pool.tile([128, T], BF16, tag=f"qkt{s}", name=f"qkt{s}")
           for s in range(NQK // 128)]
    vau_pool = ctx.enter_context(tc.tile_pool(name="vau", bufs=1))
    # [j, h, 0:64] = ones (denominator replicator; base-0 so the
    # custom-DVE reciprocal reads PSUM partitions 0..63 -- a shifted
    # base corrupts InstCustomDveAnt); [j, h, 64:128] = v_h
    vau = [vau_pool.tile([128, HL, 2 * DH], BF16, tag=f"v{tt}",
                         name=f"vau{tt}")
           for tt in range(TT)]
    for tt in range(TT):
        nc.gpsimd.memset(vau[tt][:, :, 0:DH], 1.0)
    yt_pool = ctx.enter_context(tc.tile_pool(name="yt", bufs=1))
    yt = [yt_pool.tile([128, T], BF16, tag=f"yt{s}", name=f"yt{s}")
          for s in range(NV // 128)]

    # x^T strips stay resident (late qkv filler chains still read them)
    xt_pool = ctx.enter_context(tc.tile_pool(name="xt", bufs=1))
    xt = [xt_pool.tile([128, T], BF16, tag=f"xt{s}", name=f"xt{s}")
          for s in range(CS)]

    # ---- phase 1: x^T strips (x is pre-transposed host-side) ----
    for s in range(CS):
        for h in range(2):   # halves spread across more DMA queues
            nc.sync.dma_start(xt[s][:, h * 1024:(h + 1) * 1024],
                              x[s * 128:(s + 1) * 128,
                                h * 1024:(h + 1) * 1024])
    # b_attn per-partition bias columns for the q|k strips
    bias_qk = const_pool.tile([128, 8], F32, tag="biasqk")
    nc.sync.dma_start(bias_qk[:], bqk.rearrange("(s p) -> p s", p=128))

    # ---- phases 2+3: qkv / attention / proj, filler-interleaved ----
    # PE executes in emission order, so the PE-dense qkv chains and the
    # ACT-paced attention must be woven together: attention runs
    # hp-outer (head-pair hp needs only strips hp, 4+hp), and a filler
    # queue drips one qkv chain / vau tile / proj chunk between
    # attention j-tiles. All chain psums share one 3-slot pool; the
    # psy pool (2 slots) doubles as proj psum. 8 banks total.
    wnn_pool = ctx.enter_context(tc.tile_pool(name="wnn", bufs=1))
    wn = [wnn_pool.tile([128, CS_AUG, 128], BF16, tag=f"wnn{nn}",
                        name=f"wnn{nn}")
          for nn in range(8)]
    wv_pool = ctx.enter_context(tc.tile_pool(name="wv", bufs=1))
    wv = wv_pool.tile([128, CS_AUG, NV], BF16, tag="wv")
    # DMA priority: the first chains need wn0, wn4 and wv; wp is only
    # needed ~200us in
    def wn_dma(nn):
        # per-K-strip: chain matmul s only waits for its own strip
        for s in range(CS_AUG):
            nc.sync.dma_start(
                wn[nn][:, s, :],
                wqkv[s * 128:(s + 1) * 128, nn * 128:(nn + 1) * 128])

    for nn in (0, 4):
        wn_dma(nn)
    for s in range(CS_AUG):
        nc.sync.dma_start(wv[:, s, :], wqkv[s * 128:(s + 1) * 128, NQK:])
    for nn in (1, 5, 2, 6, 3, 7):
        wn_dma(nn)
    wp_pool = ctx.enter_context(tc.tile_pool(name="wp", bufs=1))
    wp = wp_pool.tile([128, NV // 128, C], BF16, tag="wp")
    nc.sync.dma_start(wp[:], wproj.rearrange("(s p) n -> p s n", p=128))

    pt_sb_pool = ctx.enter_context(tc.tile_pool(name="ptile", bufs=4))
    n_sb_pool = ctx.enter_context(tc.tile_pool(name="ntile", bufs=2))
    osb_pool = ctx.enter_context(tc.tile_pool(name="osb", bufs=2))
    ps_s_pool = ctx.enter_context(
        tc.tile_pool(name="ps_s", bufs=3, space="PSUM"))
    ps_y_pool = ctx.enter_context(
        tc.tile_pool(name="ps_y", bufs=1, space="PSUM"))

    def qk_chain(nn, tb):
        # one [128, 512] block of qkT strip nn (borrows a pss slot)
        ps = ps_s_pool.tile([128, 2, 512], F32, tag="pss",
                            name=f"pqk{nn}_{tb}")
        for s in range(CS):
            nc.tensor.matmul(ps[:, 0, :], wn[nn][:, s, :],
                             xt[s][:, tb * 512:(tb + 1) * 512],
                             start=(s == 0), stop=(s == CS - 1))
        # evacuate with the b_attn bias folded in (per-partition)
        nc.vector.tensor_scalar_add(
            qkt[nn][:, tb * 512:(tb + 1) * 512], ps[:, 0, :],
            bias_qk[:, nn:nn + 1])

    def vau_tile(tt):
        ps = ps_s_pool.tile([128, 2, 512], F32, tag="pss",
                            name=f"pv{tt}")
        for s in range(CS_AUG):
            lhsT = (ones_strip[:, 0:128] if s == CS
                    else xt[s][:, tt * 128:(tt + 1) * 128])
            nc.tensor.matmul(ps[:, 0, :], lhsT, wv[:, s, :],
                             start=(s == 0), stop=(s == CS_AUG - 1))
        nc.vector.tensor_copy(
            vau[tt][:, :, DH:],
            ps[:, 0, :].rearrange("p (h d) -> p h d", d=DH))

    osb = [osb_pool.tile([128, C], F32, tag=f"osb{i}", name=f"osb{i}")
           for i in range(2)]

    def proj_chunk(tt, nb):
        ps = ps_s_pool.tile([128, 2, 512], F32, tag="pss",
                            name=f"po{tt}_{nb}")
        for s in range(NV // 128):
            nc.tensor.matmul(
                ps[:, 0, :],
                yt[s][:, tt * 128:(tt + 1) * 128],
                wp[:, s, nb * 512:(nb + 1) * 512],
                start=(s == 0), stop=(s == NV // 128 - 1))
        o_sb = osb[tt % 2]
        nc.vector.tensor_copy(o_sb[:, nb * 512:(nb + 1) * 512],
                              ps[:, 0, :])
        if nb == C // 512 - 1:
            nc.sync.dma_start(out[tt * 128:(tt + 1) * 128, :], o_sb[:])

    def attn_gen(ib, hp):
        # generator: yields after each j-tile so fillers can interleave
        isl = slice(ib * 512, (ib + 1) * 512)
        jmax = 4 * ib + 3
        qs = qkt[hp]              # q strip: heads (2hp, 2hp+1)
        ks = qkt[4 + hp]          # k strip
        ps_y = [ps_y_pool.tile([128, 512], F32, tag=f"psy{u}",
                               name=f"psy{u}_{hp}_{ib}")
                for u in range(2)]

        def s_exp(jj):
            off = max(0, 128 * (jj - 4 * ib))
            ps_s = ps_s_pool.tile([128, 2, 512], F32, tag="pss")
            for u in range(2):   # head-pair halves: base 0 / 64
                plo = 64 * u
                nc.tensor.matmul(
                    ps_s[:, u, off:],
                    ks[plo:plo + DH, jj * 128:(jj + 1) * 128],
                    qs[plo:plo + DH, ib * 512 + off:(ib + 1) * 512],
                    start=True, stop=True)
            p = pt_sb_pool.tile([128, 2, 512], BF16, tag="pt")
            nc.scalar.activation(p[:, :, off:], ps_s[:, :, off:],
                                 ACT_EXP, scale=SCALE)
            if jj >= 4 * ib:       # diagonal tile: zero i < j
                nc.vector.tensor_mul(
                    p[:, :, off:off + 128],
                    p[:, :, off:off + 128],
                    mask01[:, None, :].broadcast_to([128, 2, 128]))
            return p

        def pv(jj, p):
            off = max(0, 128 * (jj - 4 * ib))
            for u in range(2):
                nc.tensor.matmul(ps_y[u][:, off:],
                                 vau[jj][:, 2 * hp + u, :],
                                 p[:, u, off:],
                                 start=(jj == 0), stop=(jj == jmax))

        # software pipeline, skew 2: S(jj+2) issues before PV(jj)
        p0 = s_exp(0)
        yield
        p1 = s_exp(1)
        yield
        for jj in range(2, jmax + 1):
            p2 = s_exp(jj)
            pv(jj - 2, p0)
            p0, p1 = p1, p2
            yield
        pv(jmax - 1, p0)
        pv(jmax, p1)
        for u in range(2):
            plo = 64 * u
            rbb = n_sb_pool.tile([64, 512], F32, tag=f"rbb{u}")
            nc.vector.reciprocal_approx_fast(
                out=rbb[:], in_=ps_y[u][0:64, :])
            nc.vector.tensor_mul(yt[hp][plo:plo + DH, isl],
                                 ps_y[u][64:128, :], rbb[:])

    # lead-in: strips for hp=0 + the first vau tiles (ACT idle anyway)
    for tb in range(TB):
        qk_chain(0, tb)
    for tb in range(TB):
        qk_chain(4, tb)
    for tt in range(0, 4):
        vau_tile(tt)

    # filler queue, ordered by when attention first needs each item
    fillers = []
    for tt in range(4, TT):
        fillers.append(lambda tt=tt: vau_tile(tt))
    for hp_next in (1, 2, 3):
        for nn in (hp_next, 4 + hp_next):
            for tb in range(TB):
                fillers.append(lambda nn=nn, tb=tb: qk_chain(nn, tb))

    fi = 0
    tick = 0
    for hp in range(HL // 2):
        for ib in range(TB):
            rate = 1 if hp == HL // 2 - 1 else 2
            for _ in attn_gen(ib, hp):
                tick += 1
                if tick % rate == 0 and fi < len(fillers):
                    fillers[fi]()
                    fi += 1
            if hp == HL // 2 - 1:
                # this i-block's projection unlocks once hp3 finishes it
                for tt in range(4 * ib, 4 * ib + 4):
                    for nb in range(C // 512):
                        fillers.append(
                            lambda tt=tt, nb=nb: proj_chunk(tt, nb))
    while fi < len(fillers):   # drain: remaining proj chunks
        fillers[fi]()
        fi += 1

_BUILD_LOCK = threading.Lock()
_CACHED = {}


def build_nc(repeat=1):
    with _BUILD_LOCK:
        if repeat in _CACHED:
            return _CACHED[repeat]
        nc = bacc.Bacc("TRN2", debug=False)
        x = nc.dram_tensor("x", [C, T], BF16, kind="ExternalInput").ap()
        wqkv = nc.dram_tensor("wqkv", [CS_AUG * 128, 3 * NV], BF16,
                              kind="ExternalInput").ap()
        wproj = nc.dram_tensor("wproj", [NV, C], BF16,
                               kind="ExternalInput").ap()
        bqk = nc.dram_tensor("bqk", [NQK], F32, kind="ExternalInput").ap()
        out = nc.dram_tensor("out", [T, C], F32, kind="ExternalOutput").ap()
        with tile.TileContext(nc, pool_alloc_mode="queue") as tc:
            for _ in range(repeat):
                with ExitStack() as ctx:
                    build_attention_kernel(ctx, tc, x, wqkv, wproj, bqk, out)
        nc.compile()
        _CACHED[repeat] = nc
        return nc


def shard_inputs(x, w_attn, b_attn, w_proj, b_proj):
    """Build the per-core input maps (numpy, bf16)."""
    x = np.asarray(x, dtype=np.float32)
    w_attn = np.asarray(w_attn, dtype=np.float32)
    b_attn = np.asarray(b_attn, dtype=np.float32)
    w_proj = np.asarray(w_proj, dtype=np.float32)
    in_maps = []
    for c in range(N_CORES):
        b, hh = divmod(c, 2)
        cols = np.r_[hh * 512:(hh + 1) * 512,
                     C + hh * 512:C + (hh + 1) * 512,
                     2 * C + hh * 512:2 * C + (hh + 1) * 512]
        w_aug = np.zeros((CS_AUG * 128, 3 * NV), np.float32)
        w_aug[:C] = w_attn[:, cols]
        w_aug[C] = b_attn[cols]
        in_maps.append({
            "x": np.ascontiguousarray(x[b].T).astype(NP_BF16),
            "wqkv": w_aug.astype(NP_BF16),
            "wproj": np.ascontiguousarray(
                w_proj[hh * 512:(hh + 1) * 512]).astype(NP_BF16),
            "bqk": np.ascontiguousarray(b_attn[cols[:NQK]]),
        })
    return in_maps


def kernel(x, w_attn, b_attn, w_proj, b_proj, _profile=False, _tmpdir=None):
    nc = build_nc()
    in_maps = shard_inputs(x, w_attn, b_attn, w_proj, b_proj)
    res = run_bass_kernel_spmd(nc, in_maps, list(range(N_CORES)),
                               trace=_profile, tmpdir=_tmpdir)
    b_proj = np.asarray(b_proj, dtype=np.float32)
    out = np.empty((B, T, C), np.float32)
    for b in range(B):
        out[b] = res.results[2 * b]["out"] + res.results[2 * b + 1]["out"] \
            + b_proj[None, :]
    if _profile:
        return out, res
    return out
